# revision 13
# baseline (speedup 1.0000x reference)
# Multi-head causal self-attention (B=2, S=2048, D=1024, H=16, Dh=64) on 8
# Trainium2 NeuronCores.
#
# Sharding: core i -> (batch b = i // 4, head-group g = i % 4). Each core
# computes attention for its batch's 4 heads (feature columns 256g:256g+256 of
# the QKV projections, rows 256g:256g+256 of Wo) and produces a partial
# out-projection [S, D]. Host sums the 4 partials per batch and adds bo.
#
# MODE selects matmul operand precision:
#   "bf16": operands bf16 (fp32 PSUM accumulation), x^T loaded straight from
#           DRAM via 2-byte DMA-transpose. Fastest; ~3e-3 rel error.
#   "f32r": operands float32r (PE keeps more mantissa, 1.5 cycles/row). x is
#           shipped as a bf16 hi/lo pair, DMA-transposed, and recombined on
#           DVE into f32r x^T. ~3e-4 rel error.
#
# Per-core dataflow:
#   1. xT chunk [D, 512] via DMA-transpose.
#   2. QT = Wq_s^T xT + bq [256, S] (features on partitions), same for KT.
#      V = xT^T Wv_s + bv  [S, 256] (seq on partitions), stored augmented with
#      a ones column per head ([V_h | 1]) so the attention matmul also
#      accumulates the softmax denominator.
#   3. per (head, q-chunk): S^T tile = K_h Q_h^T [k, q]; E = exp(S^T) (scores
#      pre-scaled by 1/sqrt(Dh) via host-side Wq scaling; magnitudes are small
#      enough that max-subtraction is unnecessary); causality = skip k>q tiles
#      + triangular mask multiply on diagonal blocks; [ctx^T; denom] +=
#      [V_h | 1]^T E.
#   4. normalize (deferred two heads to keep PE fed): recip(denom) via the
#      fast Newton-Raphson DVE op, broadcast across partitions via K=1
#      matmul, multiply.
#   5. out_partial = ctxT^T Wo_s, DMA out.

import numpy as np
import ml_dtypes

import concourse.bass as bass
import concourse.mybir as mybir
import concourse.tile as tile
from concourse import bacc
from concourse.bass_utils import run_bass_kernel_spmd
from concourse.masks import make_upper_triangular

F32 = mybir.dt.float32
F32R = mybir.dt.float32r
BF16 = mybir.dt.bfloat16

MODE = "bf16"            # "bf16" | "f32r"

B, S, D = 2, 2048, 1024
H, DH = 16, 64
NCORES = 8
GROUPS = 4               # head-groups (tensor parallel)
HG = H // GROUPS         # 4 heads per group
FEAT = HG * DH           # 256 features per group
SCALE = 1.0 / 8.0        # 1/sqrt(DH), folded into Wq/bq on host

CHUNK = 512              # seq chunk (PSUM bank = 512 fp32)
NSUB = CHUNK // 128      # 4 seq subtiles per chunk
NCHUNK = S // CHUNK      # 4
KD = D // 128            # 8 k-tiles over D
MT = FEAT // 128         # 2 feature M-tiles per group


def _emit(tc, mode):
    nc = tc.nc
    WDT = BF16 if mode == "bf16" else F32R
    if mode == "bf16":
        x = nc.dram_tensor("x", [S, D], BF16, kind="ExternalInput").ap()
    else:
        x_hi = nc.dram_tensor("x_hi", [S, D], BF16, kind="ExternalInput").ap()
        x_lo = nc.dram_tensor("x_lo", [S, D], BF16, kind="ExternalInput").ap()
    wq = nc.dram_tensor("wq", [D, FEAT], WDT, kind="ExternalInput").ap()
    wk = nc.dram_tensor("wk", [D, FEAT], WDT, kind="ExternalInput").ap()
    wv = nc.dram_tensor("wv", [D, FEAT], WDT, kind="ExternalInput").ap()
    bq = nc.dram_tensor("bq", [FEAT], F32, kind="ExternalInput").ap()
    bk = nc.dram_tensor("bk", [FEAT], F32, kind="ExternalInput").ap()
    bv = nc.dram_tensor("bv", [FEAT], F32, kind="ExternalInput").ap()
    wo = nc.dram_tensor("wo", [FEAT, D], WDT, kind="ExternalInput").ap()
    out = nc.dram_tensor("out", [S, D], F32, kind="ExternalOutput").ap()

    consts = tc.alloc_tile_pool(name="consts", bufs=1)
    weights = tc.alloc_tile_pool(name="weights", bufs=1)
    persist = tc.alloc_tile_pool(name="persist", bufs=1)
    xt_pool = tc.alloc_tile_pool(name="xt", bufs=2)
    et_pool = tc.alloc_tile_pool(name="et", bufs=6)
    rc_pool = tc.alloc_tile_pool(name="rc", bufs=3)
    ob_pool = tc.alloc_tile_pool(name="ob", bufs=2)
    work_ps = tc.alloc_tile_pool(name="work_ps", bufs=5, space="PSUM")
    cx_ps = tc.alloc_tile_pool(name="cx_ps", bufs=3, space="PSUM")

    # constants
    onesf = consts.tile([128, 64], F32)   # f32 scratch (memset can't write f32r)
    nc.vector.memset(onesf, 1.0)
    ones64 = consts.tile([1, 64], F32R)
    nc.vector.tensor_copy(ones64, onesf[0:1, :])
    # tri[k, q] = 1 if q >= k else 0 (f32r memset is unsupported -> f32 there)
    tri = consts.tile([128, 128], BF16 if mode == "bf16" else F32)
    make_upper_triangular(nc, tri, val=1.0, diag=True)

    # weights
    wq_sb = weights.tile([128, KD, MT, 128], WDT)
    nc.sync.dma_start(wq_sb, wq.rearrange("(k p) (m f) -> p k m f", p=128, f=128))
    wk_sb = weights.tile([128, KD, MT, 128], WDT)
    nc.sync.dma_start(wk_sb, wk.rearrange("(k p) (m f) -> p k m f", p=128, f=128))
    wv_sb = weights.tile([128, KD, FEAT], WDT)
    nc.sync.dma_start(wv_sb, wv.rearrange("(k p) f -> p k f", p=128))
    wo_sb = weights.tile([128, MT, D], WDT)
    nc.sync.dma_start(wo_sb, wo.rearrange("(k p) n -> p k n", p=128))
    bqt = weights.tile([128, MT], F32)
    nc.sync.dma_start(bqt, bq.rearrange("(m p) -> p m", p=128))
    bkt = weights.tile([128, MT], F32)
    nc.sync.dma_start(bkt, bk.rearrange("(m p) -> p m", p=128))
    bvb = weights.tile([128, HG, DH], F32)
    nc.sync.dma_start(bvb, bv[None, :].to_broadcast([128, FEAT]).rearrange(
        "p (h f) -> p h f", h=HG))

    # persistent activations
    qt = persist.tile([128, MT, S], WDT)     # Q^T (features on partitions)
    kt = persist.tile([128, MT, S], WDT)     # K^T
    vaug = persist.tile([128, S // 128, HG, DH + 1], WDT)  # [V_h | 1] per head
    ctxT = persist.tile([128, MT, S], WDT)   # normalized ctx^T
    nc.vector.tensor_copy(vaug[:, :, :, DH],
                          onesf.rearrange("p (a b) -> p a b", a=S // 128))

    def normalize(c, h, cxt):
        """recip(denom) -> K=1 broadcast matmul -> scale ctx, into ctxT."""
        cs = c * CHUNK
        ht, hr = h // 2, 64 * (h % 2)
        rc = rc_pool.tile([1, CHUNK], F32R, tag="rc")
        with nc.allow_low_precision(reason="recip of softmax denominator"):
            nc.vector.reciprocal(rc, cxt[DH:DH + 1, :])
        bc = work_ps.tile([128, CHUNK], F32, tag="w")
        nc.tensor.matmul(bc[0:64, :], ones64, rc)
        # DVE tensor_tensor rejects two PSUM inputs; stage bc in SBUF
        bcs = rc_pool.tile([64, CHUNK], F32, tag="bcs")
        nc.scalar.copy(bcs, bc[0:64, :])
        nc.vector.tensor_mul(ctxT[hr:hr + 64, ht, cs:cs + CHUNK],
                             cxt[0:DH, :], bcs)

    def outproj(c):
        for t in range(NSUB):
            gt = c * NSUB + t
            ob = ob_pool.tile([128, D], F32)
            for n in range(D // 512):
                op = work_ps.tile([128, CHUNK], F32, tag="w")
                for k in range(MT):
                    nc.tensor.matmul(
                        op,
                        ctxT[:, k, gt * 128:(gt + 1) * 128],
                        wo_sb[:, k, 512 * n:512 * (n + 1)],
                        start=(k == 0), stop=(k == MT - 1))
                nc.vector.tensor_copy(ob[:, 512 * n:512 * (n + 1)], op)
            nc.sync.dma_start(out[gt * 128:(gt + 1) * 128, :], ob)

    pending_norm = []
    for c in range(NCHUNK):
        cs = c * CHUNK
        # ---- load x^T chunk via DMA transpose ----
        if mode == "bf16":
            xt = xt_pool.tile([128, KD, CHUNK], BF16, tag="xt")
            for k in range(KD):
                nc.sync.dma_start_transpose(
                    xt[:, k, :], x[cs:cs + CHUNK, 128 * k:128 * (k + 1)])
        else:
            xh = xt_pool.tile([128, KD, CHUNK], BF16, tag="xh")
            xl = xt_pool.tile([128, KD, CHUNK], BF16, tag="xl")
            for k in range(KD):
                nc.sync.dma_start_transpose(
                    xh[:, k, :], x_hi[cs:cs + CHUNK, 128 * k:128 * (k + 1)])
                nc.sync.dma_start_transpose(
                    xl[:, k, :], x_lo[cs:cs + CHUNK, 128 * k:128 * (k + 1)])
            xt = xt_pool.tile([128, KD, CHUNK], F32R, tag="xt")
            for k in range(KD):
                nc.vector.tensor_add(xt[:, k, :], xh[:, k, :], xl[:, k, :])

        # ---- QT / KT projections (features on partitions) ----
        for w_sb, bias_t, dst in ((wq_sb, bqt, qt), (wk_sb, bkt, kt)):
            for m in range(MT):
                ps = work_ps.tile([128, CHUNK], F32, tag="w")
                for k in range(KD):
                    nc.tensor.matmul(ps, w_sb[:, k, m, :], xt[:, k, :],
                                     start=(k == 0), stop=(k == KD - 1))
                nc.scalar.activation(
                    dst[:, m, cs:cs + CHUNK], ps,
                    mybir.ActivationFunctionType.Identity,
                    bias=bias_t[:, m:m + 1], scale=1.0)

        # ---- V projection (seq on partitions), augmented with ones col ----
        for t in range(NSUB):
            gt = c * NSUB + t
            ps = work_ps.tile([128, CHUNK], F32, tag="w")
            for k in range(KD):
                nc.tensor.matmul(ps[:, 0:FEAT],
                                 xt[:, k, t * 128:(t + 1) * 128],
                                 wv_sb[:, k, :],
                                 start=(k == 0), stop=(k == KD - 1))
            nc.vector.tensor_add(
                vaug[:, gt, :, 0:DH],
                ps[:, 0:FEAT].rearrange("p (h f) -> p h f", h=HG), bvb)

        # ---- attention for q-chunk c ----
        # Normalizes are deferred two heads (across chunk boundaries) and
        # each chunk's out-projection is deferred into the next chunk, so the
        # recip->broadcast->scale chain never stalls the PE.
        for h in range(HG):
            ht, hr = h // 2, 64 * (h % 2)
            jmax = c * NSUB + NSUB - 1
            cx = cx_ps.tile([DH + 1, CHUNK], F32, tag="cx")
            for j in range(jmax + 1):
                lv = max(0, 128 * j - cs)   # first valid q (chunk-local)
                nq = CHUNK - lv
                sp = work_ps.tile([128, CHUNK], F32, tag="w")
                nc.tensor.matmul(sp[:, 0:nq],
                                 kt[hr:hr + 64, ht, 128 * j:128 * (j + 1)],
                                 qt[hr:hr + 64, ht, cs + lv:cs + CHUNK])
                et = et_pool.tile([128, CHUNK], WDT)
                nc.scalar.activation(et[:, 0:nq], sp[:, 0:nq],
                                     mybir.ActivationFunctionType.Exp)
                if j >= c * NSUB:  # diagonal block: causal triangular mask
                    nc.vector.tensor_mul(et[:, 0:128], et[:, 0:128], tri)
                nc.tensor.matmul(cx[:, lv:CHUNK], vaug[:, j, h, :],
                                 et[:, 0:nq],
                                 start=(j == 0), stop=(j == jmax))
            pending_norm.append((c, h, cx))
            if len(pending_norm) > 2:
                normalize(*pending_norm.pop(0))
            if h == 1 and c > 0:
                outproj(c - 1)

    for cn in pending_norm:
        normalize(*cn)
    outproj(NCHUNK - 1)

    for p in (cx_ps, work_ps, ob_pool, rc_pool, et_pool, xt_pool,
              persist, weights, consts):
        p.release()


_BUILT = None


def _build():
    global _BUILT
    if _BUILT is None:
        nc = bacc.Bacc("TRN2", target_bir_lowering=False, debug=False,
                       num_devices=NCORES)
        with tile.TileContext(nc) as tc:
            _emit(tc, MODE)
        nc.compile()
        _BUILT = nc
    return _BUILT


def _bf16(a):
    return np.ascontiguousarray(np.asarray(a, dtype=np.float32)).astype(
        ml_dtypes.bfloat16)


def _f32(a):
    return np.ascontiguousarray(np.asarray(a, dtype=np.float32))


def _shards(inputs):
    x = np.asarray(inputs["x"], dtype=np.float32)
    maps = []
    for core in range(NCORES):
        b, g = core // GROUPS, core % GROUPS
        f0 = g * FEAT
        m = {
            "bq": _f32(np.asarray(inputs["bq"], np.float32)[f0:f0 + FEAT] * SCALE),
            "bk": _f32(np.asarray(inputs["bk"], np.float32)[f0:f0 + FEAT]),
            "bv": _f32(np.asarray(inputs["bv"], np.float32)[f0:f0 + FEAT]),
        }
        wq_s = np.asarray(inputs["Wq"], np.float32)[:, f0:f0 + FEAT] * SCALE
        wk_s = np.asarray(inputs["Wk"], np.float32)[:, f0:f0 + FEAT]
        wv_s = np.asarray(inputs["Wv"], np.float32)[:, f0:f0 + FEAT]
        wo_s = np.asarray(inputs["Wo"], np.float32)[f0:f0 + FEAT, :]
        if MODE == "bf16":
            m["x"] = _bf16(x[b])
            m["wq"], m["wk"], m["wv"], m["wo"] = (
                _bf16(wq_s), _bf16(wk_s), _bf16(wv_s), _bf16(wo_s))
        else:
            hi = x[b].astype(ml_dtypes.bfloat16)
            lo = (x[b] - hi.astype(np.float32)).astype(ml_dtypes.bfloat16)
            m["x_hi"] = np.ascontiguousarray(hi)
            m["x_lo"] = np.ascontiguousarray(lo)
            m["wq"], m["wk"], m["wv"], m["wo"] = (
                _f32(wq_s), _f32(wk_s), _f32(wv_s), _f32(wo_s))
        maps.append(m)
    return maps


def kernel(trace=False, **inputs):
    nc = _build()
    res = run_bass_kernel_spmd(nc, _shards(inputs), core_ids=list(range(NCORES)),
                               trace=trace)
    partial = np.stack([r_["out"] for r_ in res.results])  # [8, S, D]
    acc = partial.reshape(B, GROUPS, S, D).astype(np.float64).sum(axis=1)
    acc += np.asarray(inputs["bo"], dtype=np.float64)
    out = acc.astype(np.float32)
    if trace:
        return out, res
    return out


# revision 14
# speedup vs baseline: 1.0730x; 1.0730x over previous
# Multi-head causal self-attention (B=2, S=2048, D=1024, H=16, Dh=64) on 8
# Trainium2 NeuronCores.
#
# Sharding: core i -> (batch b = i // 4, head-group g = i % 4). Each core
# computes attention for its batch's 4 heads (feature columns 256g:256g+256 of
# the QKV projections, rows 256g:256g+256 of Wo) and produces a partial
# out-projection [S, D]. Host sums the 4 partials per batch and adds bo.
#
# MODE selects matmul operand precision:
#   "bf16": operands bf16 (fp32 PSUM accumulation), x^T loaded straight from
#           DRAM via 2-byte DMA-transpose. Fastest; ~3e-3 rel error.
#   "f32r": operands float32r (PE keeps more mantissa, 1.5 cycles/row). x is
#           shipped as a bf16 hi/lo pair, DMA-transposed, and recombined on
#           DVE into f32r x^T. ~3e-4 rel error.
#
# Per-core dataflow:
#   1. xT chunk [D, 512] via DMA-transpose.
#   2. QT = Wq_s^T xT + bq [256, S] (features on partitions), same for KT.
#      V = xT^T Wv_s + bv  [S, 256] (seq on partitions), stored augmented with
#      a ones column per head ([V_h | 1]) so the attention matmul also
#      accumulates the softmax denominator.
#   3. per (head, q-chunk): S^T tile = K_h Q_h^T [k, q]; E = exp(S^T) (scores
#      pre-scaled by 1/sqrt(Dh) via host-side Wq scaling; magnitudes are small
#      enough that max-subtraction is unnecessary); causality = skip k>q tiles
#      + triangular mask multiply on diagonal blocks; [ctx^T; denom] +=
#      [V_h | 1]^T E.
#   4. normalize (deferred two heads to keep PE fed): recip(denom) via the
#      fast Newton-Raphson DVE op, broadcast across partitions via K=1
#      matmul, multiply.
#   5. out_partial = ctxT^T Wo_s, DMA out.

import numpy as np
import ml_dtypes

import concourse.bass as bass
import concourse.mybir as mybir
import concourse.tile as tile
from concourse import bacc
from concourse.bass_utils import run_bass_kernel_spmd
from concourse.masks import make_upper_triangular

F32 = mybir.dt.float32
F32R = mybir.dt.float32r
BF16 = mybir.dt.bfloat16

MODE = "bf16"            # "bf16" | "f32r"

B, S, D = 2, 2048, 1024
H, DH = 16, 64
NCORES = 8
GROUPS = 4               # head-groups (tensor parallel)
HG = H // GROUPS         # 4 heads per group
FEAT = HG * DH           # 256 features per group
SCALE = 1.0 / 8.0        # 1/sqrt(DH), folded into Wq/bq on host

CHUNK = 512              # seq chunk (PSUM bank = 512 fp32)
NSUB = CHUNK // 128      # 4 seq subtiles per chunk
NCHUNK = S // CHUNK      # 4
KD = D // 128            # 8 k-tiles over D
MT = FEAT // 128         # 2 feature M-tiles per group


def _emit(tc, mode):
    nc = tc.nc
    WDT = BF16 if mode == "bf16" else F32R
    if mode == "bf16":
        x = nc.dram_tensor("x", [S, D], BF16, kind="ExternalInput").ap()
    else:
        x_hi = nc.dram_tensor("x_hi", [S, D], BF16, kind="ExternalInput").ap()
        x_lo = nc.dram_tensor("x_lo", [S, D], BF16, kind="ExternalInput").ap()
    wq = nc.dram_tensor("wq", [D, FEAT], WDT, kind="ExternalInput").ap()
    wk = nc.dram_tensor("wk", [D, FEAT], WDT, kind="ExternalInput").ap()
    wv = nc.dram_tensor("wv", [D, FEAT], WDT, kind="ExternalInput").ap()
    bq = nc.dram_tensor("bq", [FEAT], F32, kind="ExternalInput").ap()
    bk = nc.dram_tensor("bk", [FEAT], F32, kind="ExternalInput").ap()
    bv = nc.dram_tensor("bv", [FEAT], F32, kind="ExternalInput").ap()
    wo = nc.dram_tensor("wo", [FEAT, D], WDT, kind="ExternalInput").ap()
    out = nc.dram_tensor("out", [S, D], F32, kind="ExternalOutput").ap()

    consts = tc.alloc_tile_pool(name="consts", bufs=1)
    weights = tc.alloc_tile_pool(name="weights", bufs=1)
    persist = tc.alloc_tile_pool(name="persist", bufs=1)
    xt_pool = tc.alloc_tile_pool(name="xt", bufs=2)
    et_pool = tc.alloc_tile_pool(name="et", bufs=6)
    rc_pool = tc.alloc_tile_pool(name="rc", bufs=3)
    ob_pool = tc.alloc_tile_pool(name="ob", bufs=2)
    work_ps = tc.alloc_tile_pool(name="work_ps", bufs=5, space="PSUM")
    cx_ps = tc.alloc_tile_pool(name="cx_ps", bufs=3, space="PSUM")

    # constants
    onesf = consts.tile([128, 64], F32)   # f32 scratch (memset can't write f32r)
    nc.vector.memset(onesf, 1.0)
    ones64 = consts.tile([1, 64], F32R)
    nc.vector.tensor_copy(ones64, onesf[0:1, :])
    # tri[k, q] = 1 if q >= k else 0 (f32r memset is unsupported -> f32 there)
    tri = consts.tile([128, 128], BF16 if mode == "bf16" else F32)
    make_upper_triangular(nc, tri, val=1.0, diag=True)

    # weights
    wq_sb = weights.tile([128, KD, MT, 128], WDT)
    nc.sync.dma_start(wq_sb, wq.rearrange("(k p) (m f) -> p k m f", p=128, f=128))
    wk_sb = weights.tile([128, KD, MT, 128], WDT)
    nc.sync.dma_start(wk_sb, wk.rearrange("(k p) (m f) -> p k m f", p=128, f=128))
    wv_sb = weights.tile([128, KD, FEAT], WDT)
    nc.sync.dma_start(wv_sb, wv.rearrange("(k p) f -> p k f", p=128))
    wo_sb = weights.tile([128, MT, D], WDT)
    nc.sync.dma_start(wo_sb, wo.rearrange("(k p) n -> p k n", p=128))
    bqt = weights.tile([128, MT], F32)
    nc.sync.dma_start(bqt, bq.rearrange("(m p) -> p m", p=128))
    bkt = weights.tile([128, MT], F32)
    nc.sync.dma_start(bkt, bk.rearrange("(m p) -> p m", p=128))
    bvb = weights.tile([128, HG, DH], F32)
    nc.sync.dma_start(bvb, bv[None, :].to_broadcast([128, FEAT]).rearrange(
        "p (h f) -> p h f", h=HG))

    # persistent activations
    qt = persist.tile([128, MT, S], WDT)     # Q^T (features on partitions)
    kt = persist.tile([128, MT, S], WDT)     # K^T
    vaug = persist.tile([128, S // 128, HG, DH + 1], WDT)  # [V_h | 1] per head
    ctxT = persist.tile([128, MT, S], WDT)   # normalized ctx^T
    nc.vector.tensor_copy(vaug[:, :, :, DH],
                          onesf.rearrange("p (a b) -> p a b", a=S // 128))

    def normalize(c, h, cxt):
        """recip(denom) -> K=1 broadcast matmul -> scale ctx, into ctxT."""
        cs = c * CHUNK
        ht, hr = h // 2, 64 * (h % 2)
        rc = rc_pool.tile([1, CHUNK], F32R, tag="rc")
        with nc.allow_low_precision(reason="recip of softmax denominator"):
            nc.vector.reciprocal(rc, cxt[DH:DH + 1, :])
        bc = work_ps.tile([128, CHUNK], F32, tag="w")
        nc.tensor.matmul(bc[0:64, :], ones64, rc)
        # DVE tensor_tensor rejects two PSUM inputs; stage bc in SBUF
        bcs = rc_pool.tile([64, CHUNK], F32, tag="bcs")
        nc.scalar.copy(bcs, bc[0:64, :])
        nc.vector.tensor_mul(ctxT[hr:hr + 64, ht, cs:cs + CHUNK],
                             cxt[0:DH, :], bcs)

    def outproj(c):
        for t in range(NSUB):
            gt = c * NSUB + t
            ob = ob_pool.tile([128, D], F32)
            for n in range(D // 512):
                op = work_ps.tile([128, CHUNK], F32, tag="w")
                for k in range(MT):
                    nc.tensor.matmul(
                        op,
                        ctxT[:, k, gt * 128:(gt + 1) * 128],
                        wo_sb[:, k, 512 * n:512 * (n + 1)],
                        start=(k == 0), stop=(k == MT - 1))
                nc.vector.tensor_copy(ob[:, 512 * n:512 * (n + 1)], op)
            nc.sync.dma_start(out[gt * 128:(gt + 1) * 128, :], ob)

    pending_norm = []
    for c in range(NCHUNK):
        cs = c * CHUNK
        # ---- load x^T chunk via DMA transpose ----
        if mode == "bf16":
            xt = xt_pool.tile([128, KD, CHUNK], BF16, tag="xt")
            for k in range(KD):
                nc.sync.dma_start_transpose(
                    xt[:, k, :], x[cs:cs + CHUNK, 128 * k:128 * (k + 1)])
        else:
            xh = xt_pool.tile([128, KD, CHUNK], BF16, tag="xh")
            xl = xt_pool.tile([128, KD, CHUNK], BF16, tag="xl")
            for k in range(KD):
                nc.sync.dma_start_transpose(
                    xh[:, k, :], x_hi[cs:cs + CHUNK, 128 * k:128 * (k + 1)])
                nc.sync.dma_start_transpose(
                    xl[:, k, :], x_lo[cs:cs + CHUNK, 128 * k:128 * (k + 1)])
            xt = xt_pool.tile([128, KD, CHUNK], F32R, tag="xt")
            for k in range(KD):
                nc.vector.tensor_add(xt[:, k, :], xh[:, k, :], xl[:, k, :])

        # ---- QT / KT projections (features on partitions) ----
        for w_sb, bias_t, dst in ((wq_sb, bqt, qt), (wk_sb, bkt, kt)):
            for m in range(MT):
                ps = work_ps.tile([128, CHUNK], F32, tag="w")
                for k in range(KD):
                    nc.tensor.matmul(ps, w_sb[:, k, m, :], xt[:, k, :],
                                     start=(k == 0), stop=(k == KD - 1))
                nc.scalar.activation(
                    dst[:, m, cs:cs + CHUNK], ps,
                    mybir.ActivationFunctionType.Identity,
                    bias=bias_t[:, m:m + 1], scale=1.0)

        # ---- V projection (seq on partitions), augmented with ones col ----
        for t in range(NSUB):
            gt = c * NSUB + t
            ps = work_ps.tile([128, CHUNK], F32, tag="w")
            for k in range(KD):
                nc.tensor.matmul(ps[:, 0:FEAT],
                                 xt[:, k, t * 128:(t + 1) * 128],
                                 wv_sb[:, k, :],
                                 start=(k == 0), stop=(k == KD - 1))
            nc.vector.tensor_add(
                vaug[:, gt, :, 0:DH],
                ps[:, 0:FEAT].rearrange("p (h f) -> p h f", h=HG), bvb)

        # ---- attention for q-chunk c ----
        # Normalizes are deferred two heads (across chunk boundaries) and
        # each chunk's out-projection is deferred into the next chunk, so the
        # recip->broadcast->scale chain never stalls the PE.
        for h in range(HG):
            ht, hr = h // 2, 64 * (h % 2)
            jmax = c * NSUB + NSUB - 1
            cx = cx_ps.tile([DH + 1, CHUNK], F32, tag="cx")
            for j in range(jmax + 1):
                lv = max(0, 128 * j - cs)   # first valid q (chunk-local)
                nq = CHUNK - lv
                sp = work_ps.tile([128, CHUNK], F32, tag="w")
                nc.tensor.matmul(sp[:, 0:nq],
                                 kt[hr:hr + 64, ht, 128 * j:128 * (j + 1)],
                                 qt[hr:hr + 64, ht, cs + lv:cs + CHUNK])
                et = et_pool.tile([128, CHUNK], WDT)
                nc.scalar.activation(et[:, 0:nq], sp[:, 0:nq],
                                     mybir.ActivationFunctionType.Exp)
                if j >= c * NSUB:  # diagonal block: causal triangular mask
                    nc.vector.tensor_mul(et[:, 0:128], et[:, 0:128], tri)
                nc.tensor.matmul(cx[:, lv:CHUNK], vaug[:, j, h, :],
                                 et[:, 0:nq],
                                 start=(j == 0), stop=(j == jmax))
            pending_norm.append((c, h, cx))
            if len(pending_norm) > 2:
                normalize(*pending_norm.pop(0))
            if h == 2 and c > 0:
                outproj(c - 1)

    for cn in pending_norm:
        normalize(*cn)
    outproj(NCHUNK - 1)

    for p in (cx_ps, work_ps, ob_pool, rc_pool, et_pool, xt_pool,
              persist, weights, consts):
        p.release()


_BUILT = None


def _build():
    global _BUILT
    if _BUILT is None:
        nc = bacc.Bacc("TRN2", target_bir_lowering=False, debug=False,
                       num_devices=NCORES)
        with tile.TileContext(nc) as tc:
            _emit(tc, MODE)
        nc.compile()
        _BUILT = nc
    return _BUILT


def _bf16(a):
    return np.ascontiguousarray(np.asarray(a, dtype=np.float32)).astype(
        ml_dtypes.bfloat16)


def _f32(a):
    return np.ascontiguousarray(np.asarray(a, dtype=np.float32))


def _shards(inputs):
    x = np.asarray(inputs["x"], dtype=np.float32)
    maps = []
    for core in range(NCORES):
        b, g = core // GROUPS, core % GROUPS
        f0 = g * FEAT
        m = {
            "bq": _f32(np.asarray(inputs["bq"], np.float32)[f0:f0 + FEAT] * SCALE),
            "bk": _f32(np.asarray(inputs["bk"], np.float32)[f0:f0 + FEAT]),
            "bv": _f32(np.asarray(inputs["bv"], np.float32)[f0:f0 + FEAT]),
        }
        wq_s = np.asarray(inputs["Wq"], np.float32)[:, f0:f0 + FEAT] * SCALE
        wk_s = np.asarray(inputs["Wk"], np.float32)[:, f0:f0 + FEAT]
        wv_s = np.asarray(inputs["Wv"], np.float32)[:, f0:f0 + FEAT]
        wo_s = np.asarray(inputs["Wo"], np.float32)[f0:f0 + FEAT, :]
        if MODE == "bf16":
            m["x"] = _bf16(x[b])
            m["wq"], m["wk"], m["wv"], m["wo"] = (
                _bf16(wq_s), _bf16(wk_s), _bf16(wv_s), _bf16(wo_s))
        else:
            hi = x[b].astype(ml_dtypes.bfloat16)
            lo = (x[b] - hi.astype(np.float32)).astype(ml_dtypes.bfloat16)
            m["x_hi"] = np.ascontiguousarray(hi)
            m["x_lo"] = np.ascontiguousarray(lo)
            m["wq"], m["wk"], m["wv"], m["wo"] = (
                _f32(wq_s), _f32(wk_s), _f32(wv_s), _f32(wo_s))
        maps.append(m)
    return maps


def kernel(trace=False, **inputs):
    nc = _build()
    res = run_bass_kernel_spmd(nc, _shards(inputs), core_ids=list(range(NCORES)),
                               trace=trace)
    partial = np.stack([r_["out"] for r_ in res.results])  # [8, S, D]
    acc = partial.reshape(B, GROUPS, S, D).astype(np.float64).sum(axis=1)
    acc += np.asarray(inputs["bo"], dtype=np.float64)
    out = acc.astype(np.float32)
    if trace:
        return out, res
    return out


# revision 16
# speedup vs baseline: 1.2167x; 1.1340x over previous
# Multi-head causal self-attention (B=2, S=2048, D=1024, H=16, Dh=64) on 8
# Trainium2 NeuronCores.
#
# Sharding: core i -> (batch b = i // 4, head-group g = i % 4). Each core
# computes attention for its batch's 4 heads (feature columns 256g:256g+256 of
# the QKV projections, rows 256g:256g+256 of Wo) and produces a partial
# out-projection [S, D]. Host sums the 4 partials per batch and adds bo.
#
# MODE selects matmul operand precision:
#   "bf16": operands bf16 (fp32 PSUM accumulation), x^T loaded straight from
#           DRAM via 2-byte DMA-transpose. Fastest; ~3e-3 rel error.
#   "f32r": operands float32r (PE keeps more mantissa, 1.5 cycles/row). x is
#           shipped as a bf16 hi/lo pair, DMA-transposed, and recombined on
#           DVE into f32r x^T. ~3e-4 rel error.
#
# Per-core dataflow:
#   1. xT chunk [D, 512] via DMA-transpose.
#   2. QT = Wq_s^T xT + bq [256, S] (features on partitions), same for KT.
#      V = xT^T Wv_s + bv  [S, 256] (seq on partitions), stored augmented with
#      a ones column per head ([V_h | 1]) so the attention matmul also
#      accumulates the softmax denominator.
#   3. per (head, q-chunk): S^T tile = K_h Q_h^T [k, q]; E = exp(S^T) (scores
#      pre-scaled by 1/sqrt(Dh) via host-side Wq scaling; magnitudes are small
#      enough that max-subtraction is unnecessary); causality = skip k>q tiles
#      + triangular mask multiply on diagonal blocks; [ctx^T; denom] +=
#      [V_h | 1]^T E.
#   4. normalize (deferred two heads to keep PE fed): recip(denom) via the
#      fast Newton-Raphson DVE op, broadcast across partitions via K=1
#      matmul, multiply.
#   5. out_partial = ctxT^T Wo_s, DMA out.

import numpy as np
import ml_dtypes

import concourse.bass as bass
import concourse.mybir as mybir
import concourse.tile as tile
from concourse import bacc
from concourse.bass_utils import run_bass_kernel_spmd
from concourse.masks import make_upper_triangular

F32 = mybir.dt.float32
F32R = mybir.dt.float32r
BF16 = mybir.dt.bfloat16

MODE = "bf16"            # "bf16" | "f32r"

B, S, D = 2, 2048, 1024
H, DH = 16, 64
NCORES = 8
GROUPS = 4               # head-groups (tensor parallel)
HG = H // GROUPS         # 4 heads per group
FEAT = HG * DH           # 256 features per group
SCALE = 1.0 / 8.0        # 1/sqrt(DH), folded into Wq/bq on host

CHUNK = 512              # seq chunk (PSUM bank = 512 fp32)
NSUB = CHUNK // 128      # 4 seq subtiles per chunk
NCHUNK = S // CHUNK      # 4
KD = D // 128            # 8 k-tiles over D
MT = FEAT // 128         # 2 feature M-tiles per group


def _emit(tc, mode):
    nc = tc.nc
    WDT = BF16 if mode == "bf16" else F32R
    if mode == "bf16":
        x = nc.dram_tensor("x", [S, D], BF16, kind="ExternalInput").ap()
    else:
        x_hi = nc.dram_tensor("x_hi", [S, D], BF16, kind="ExternalInput").ap()
        x_lo = nc.dram_tensor("x_lo", [S, D], BF16, kind="ExternalInput").ap()
    wq = nc.dram_tensor("wq", [D, FEAT], WDT, kind="ExternalInput").ap()
    wk = nc.dram_tensor("wk", [D, FEAT], WDT, kind="ExternalInput").ap()
    wv = nc.dram_tensor("wv", [D, FEAT], WDT, kind="ExternalInput").ap()
    bq = nc.dram_tensor("bq", [FEAT], F32, kind="ExternalInput").ap()
    bk = nc.dram_tensor("bk", [FEAT], F32, kind="ExternalInput").ap()
    bv = nc.dram_tensor("bv", [FEAT], F32, kind="ExternalInput").ap()
    wo = nc.dram_tensor("wo", [FEAT, D], WDT, kind="ExternalInput").ap()
    out = nc.dram_tensor("out", [S, D], F32, kind="ExternalOutput").ap()

    consts = tc.alloc_tile_pool(name="consts", bufs=1)
    weights = tc.alloc_tile_pool(name="weights", bufs=1)
    persist = tc.alloc_tile_pool(name="persist", bufs=1)
    xt_pool = tc.alloc_tile_pool(name="xt", bufs=2)
    et_pool = tc.alloc_tile_pool(name="et", bufs=6)
    rc_pool = tc.alloc_tile_pool(name="rc", bufs=3)
    ob_pool = tc.alloc_tile_pool(name="ob", bufs=2)
    work_ps = tc.alloc_tile_pool(name="work_ps", bufs=6, space="PSUM")
    cx_ps = tc.alloc_tile_pool(name="cx_ps", bufs=2, space="PSUM")

    # constants
    onesf = consts.tile([128, 64], F32)   # f32 scratch (memset can't write f32r)
    nc.vector.memset(onesf, 1.0)
    # tri[k, q] = 1 if q >= k else 0 (f32r memset is unsupported -> f32 there)
    tri = consts.tile([128, 128], BF16 if mode == "bf16" else F32)
    make_upper_triangular(nc, tri, val=1.0, diag=True)

    # weights
    wq_sb = weights.tile([128, KD, MT, 128], WDT)
    nc.sync.dma_start(wq_sb, wq.rearrange("(k p) (m f) -> p k m f", p=128, f=128))
    wk_sb = weights.tile([128, KD, MT, 128], WDT)
    nc.sync.dma_start(wk_sb, wk.rearrange("(k p) (m f) -> p k m f", p=128, f=128))
    wv_sb = weights.tile([128, KD, FEAT], WDT)
    nc.sync.dma_start(wv_sb, wv.rearrange("(k p) f -> p k f", p=128))
    wo_sb = weights.tile([128, MT, D], WDT)
    nc.sync.dma_start(wo_sb, wo.rearrange("(k p) n -> p k n", p=128))
    bqt = weights.tile([128, MT], F32)
    nc.sync.dma_start(bqt, bq.rearrange("(m p) -> p m", p=128))
    bkt = weights.tile([128, MT], F32)
    nc.sync.dma_start(bkt, bk.rearrange("(m p) -> p m", p=128))
    bvb = weights.tile([128, HG, DH], F32)
    nc.sync.dma_start(bvb, bv[None, :].to_broadcast([128, FEAT]).rearrange(
        "p (h f) -> p h f", h=HG))

    # persistent activations
    qt = persist.tile([128, MT, S], WDT)     # Q^T (features on partitions)
    kt = persist.tile([128, MT, S], WDT)     # K^T
    vaug = persist.tile([128, S // 128, HG, DH + 1], WDT)  # [V_h | 1] per head
    ctxT = persist.tile([128, MT, S], WDT)   # normalized ctx^T
    nc.vector.tensor_copy(vaug[:, :, :, DH],
                          onesf.rearrange("p (a b) -> p a b", a=S // 128))

    def normalize(c, h, cxt):
        """recip(denom), broadcast across partitions on GPSIMD
        (keeps the PE entirely out of the normalization chain), scale ctx."""
        cs = c * CHUNK
        ht, hr = h // 2, 64 * (h % 2)
        rc = rc_pool.tile([1, CHUNK], F32, tag="rc")
        nc.vector.reciprocal(rc, cxt[DH:DH + 1, :])
        bcs = rc_pool.tile([64, CHUNK], F32, tag="bcs")
        nc.gpsimd.partition_broadcast(bcs, rc)
        nc.vector.tensor_mul(ctxT[hr:hr + 64, ht, cs:cs + CHUNK],
                             cxt[0:DH, :], bcs)

    def outproj(c):
        for t in range(NSUB):
            gt = c * NSUB + t
            ob = ob_pool.tile([128, D], F32)
            for n in range(D // 512):
                op = work_ps.tile([128, CHUNK], F32, tag="w")
                for k in range(MT):
                    nc.tensor.matmul(
                        op,
                        ctxT[:, k, gt * 128:(gt + 1) * 128],
                        wo_sb[:, k, 512 * n:512 * (n + 1)],
                        start=(k == 0), stop=(k == MT - 1))
                nc.vector.tensor_copy(ob[:, 512 * n:512 * (n + 1)], op)
            nc.sync.dma_start(out[gt * 128:(gt + 1) * 128, :], ob)

    for c in range(NCHUNK):
        cs = c * CHUNK
        # ---- load x^T chunk via DMA transpose ----
        if mode == "bf16":
            xt = xt_pool.tile([128, KD, CHUNK], BF16, tag="xt")
            for k in range(KD):
                nc.sync.dma_start_transpose(
                    xt[:, k, :], x[cs:cs + CHUNK, 128 * k:128 * (k + 1)])
        else:
            xh = xt_pool.tile([128, KD, CHUNK], BF16, tag="xh")
            xl = xt_pool.tile([128, KD, CHUNK], BF16, tag="xl")
            for k in range(KD):
                nc.sync.dma_start_transpose(
                    xh[:, k, :], x_hi[cs:cs + CHUNK, 128 * k:128 * (k + 1)])
                nc.sync.dma_start_transpose(
                    xl[:, k, :], x_lo[cs:cs + CHUNK, 128 * k:128 * (k + 1)])
            xt = xt_pool.tile([128, KD, CHUNK], F32R, tag="xt")
            for k in range(KD):
                nc.vector.tensor_add(xt[:, k, :], xh[:, k, :], xl[:, k, :])

        # ---- QT / KT projections (features on partitions) ----
        for w_sb, bias_t, dst in ((wq_sb, bqt, qt), (wk_sb, bkt, kt)):
            for m in range(MT):
                ps = work_ps.tile([128, CHUNK], F32, tag="w")
                for k in range(KD):
                    nc.tensor.matmul(ps, w_sb[:, k, m, :], xt[:, k, :],
                                     start=(k == 0), stop=(k == KD - 1))
                nc.scalar.activation(
                    dst[:, m, cs:cs + CHUNK], ps,
                    mybir.ActivationFunctionType.Identity,
                    bias=bias_t[:, m:m + 1], scale=1.0)

        # ---- V projection (seq on partitions), augmented with ones col ----
        for t in range(NSUB):
            gt = c * NSUB + t
            ps = work_ps.tile([128, CHUNK], F32, tag="w")
            for k in range(KD):
                nc.tensor.matmul(ps[:, 0:FEAT],
                                 xt[:, k, t * 128:(t + 1) * 128],
                                 wv_sb[:, k, :],
                                 start=(k == 0), stop=(k == KD - 1))
            nc.vector.tensor_add(
                vaug[:, gt, :, 0:DH],
                ps[:, 0:FEAT].rearrange("p (h f) -> p h f", h=HG), bvb)

        # ---- attention for q-chunk c ----
        # Normalizes are deferred two heads (across chunk boundaries) and
        # each chunk's out-projection is deferred into the next chunk, so the
        # recip->broadcast->scale chain never stalls the PE.
        for h in range(HG):
            ht, hr = h // 2, 64 * (h % 2)
            jmax = c * NSUB + NSUB - 1
            cx = cx_ps.tile([DH + 1, CHUNK], F32, tag="cx")
            for j in range(jmax + 1):
                lv = max(0, 128 * j - cs)   # first valid q (chunk-local)
                nq = CHUNK - lv
                sp = work_ps.tile([128, CHUNK], F32, tag="w")
                nc.tensor.matmul(sp[:, 0:nq],
                                 kt[hr:hr + 64, ht, 128 * j:128 * (j + 1)],
                                 qt[hr:hr + 64, ht, cs + lv:cs + CHUNK])
                et = et_pool.tile([128, CHUNK], WDT)
                nc.scalar.activation(et[:, 0:nq], sp[:, 0:nq],
                                     mybir.ActivationFunctionType.Exp)
                if j >= c * NSUB:  # diagonal block: causal triangular mask
                    nc.vector.tensor_mul(et[:, 0:128], et[:, 0:128], tri)
                nc.tensor.matmul(cx[:, lv:CHUNK], vaug[:, j, h, :],
                                 et[:, 0:nq],
                                 start=(j == 0), stop=(j == jmax))
            normalize(c, h, cx)
            if h == 1 and c > 0:
                outproj(c - 1)

    outproj(NCHUNK - 1)

    for p in (cx_ps, work_ps, ob_pool, rc_pool, et_pool, xt_pool,
              persist, weights, consts):
        p.release()


_BUILT = None


def _build():
    global _BUILT
    if _BUILT is None:
        nc = bacc.Bacc("TRN2", target_bir_lowering=False, debug=False,
                       num_devices=NCORES)
        with tile.TileContext(nc) as tc:
            _emit(tc, MODE)
        nc.compile()
        _BUILT = nc
    return _BUILT


def _bf16(a):
    return np.ascontiguousarray(np.asarray(a, dtype=np.float32)).astype(
        ml_dtypes.bfloat16)


def _f32(a):
    return np.ascontiguousarray(np.asarray(a, dtype=np.float32))


def _shards(inputs):
    x = np.asarray(inputs["x"], dtype=np.float32)
    maps = []
    for core in range(NCORES):
        b, g = core // GROUPS, core % GROUPS
        f0 = g * FEAT
        m = {
            "bq": _f32(np.asarray(inputs["bq"], np.float32)[f0:f0 + FEAT] * SCALE),
            "bk": _f32(np.asarray(inputs["bk"], np.float32)[f0:f0 + FEAT]),
            "bv": _f32(np.asarray(inputs["bv"], np.float32)[f0:f0 + FEAT]),
        }
        wq_s = np.asarray(inputs["Wq"], np.float32)[:, f0:f0 + FEAT] * SCALE
        wk_s = np.asarray(inputs["Wk"], np.float32)[:, f0:f0 + FEAT]
        wv_s = np.asarray(inputs["Wv"], np.float32)[:, f0:f0 + FEAT]
        wo_s = np.asarray(inputs["Wo"], np.float32)[f0:f0 + FEAT, :]
        if MODE == "bf16":
            m["x"] = _bf16(x[b])
            m["wq"], m["wk"], m["wv"], m["wo"] = (
                _bf16(wq_s), _bf16(wk_s), _bf16(wv_s), _bf16(wo_s))
        else:
            hi = x[b].astype(ml_dtypes.bfloat16)
            lo = (x[b] - hi.astype(np.float32)).astype(ml_dtypes.bfloat16)
            m["x_hi"] = np.ascontiguousarray(hi)
            m["x_lo"] = np.ascontiguousarray(lo)
            m["wq"], m["wk"], m["wv"], m["wo"] = (
                _f32(wq_s), _f32(wk_s), _f32(wv_s), _f32(wo_s))
        maps.append(m)
    return maps


def kernel(trace=False, **inputs):
    nc = _build()
    res = run_bass_kernel_spmd(nc, _shards(inputs), core_ids=list(range(NCORES)),
                               trace=trace)
    partial = np.stack([r_["out"] for r_ in res.results])  # [8, S, D]
    acc = partial.reshape(B, GROUPS, S, D).astype(np.float64).sum(axis=1)
    acc += np.asarray(inputs["bo"], dtype=np.float64)
    out = acc.astype(np.float32)
    if trace:
        return out, res
    return out


# revision 17
# speedup vs baseline: 1.4105x; 1.1593x over previous
# Multi-head causal self-attention (B=2, S=2048, D=1024, H=16, Dh=64) on 8
# Trainium2 NeuronCores.
#
# Sharding: core i -> (batch b = i // 4, head-group g = i % 4). Each core
# computes attention for its batch's 4 heads (feature columns 256g:256g+256 of
# the QKV projections, rows 256g:256g+256 of Wo) and produces a partial
# out-projection [S, D]. Host sums the 4 partials per batch and adds bo.
#
# MODE selects matmul operand precision:
#   "bf16": operands bf16 (fp32 PSUM accumulation), x^T loaded straight from
#           DRAM via 2-byte DMA-transpose. Fastest; ~3e-3 rel error.
#   "f32r": operands float32r (PE keeps more mantissa, 1.5 cycles/row). x is
#           shipped as a bf16 hi/lo pair, DMA-transposed, and recombined on
#           DVE into f32r x^T. ~3e-4 rel error.
#
# Per-core dataflow:
#   1. xT chunk [D, 512] via DMA-transpose.
#   2. QT = Wq_s^T xT + bq [256, S] (features on partitions), same for KT.
#      V = xT^T Wv_s + bv  [S, 256] (seq on partitions), stored augmented with
#      a ones column per head ([V_h | 1]) so the attention matmul also
#      accumulates the softmax denominator.
#   3. per (head, q-chunk): S^T tile = K_h Q_h^T [k, q]; E = exp(S^T) (scores
#      pre-scaled by 1/sqrt(Dh) via host-side Wq scaling; magnitudes are small
#      enough that max-subtraction is unnecessary); causality = skip k>q tiles
#      + triangular mask multiply on diagonal blocks; [ctx^T; denom] +=
#      [V_h | 1]^T E.
#   4. normalize (deferred two heads to keep PE fed): recip(denom) via the
#      fast Newton-Raphson DVE op, broadcast across partitions via K=1
#      matmul, multiply.
#   5. out_partial = ctxT^T Wo_s, DMA out.

import numpy as np
import ml_dtypes

import concourse.bass as bass
import concourse.mybir as mybir
import concourse.tile as tile
from concourse import bacc
from concourse.bass_utils import run_bass_kernel_spmd
from concourse.masks import make_upper_triangular

F32 = mybir.dt.float32
F32R = mybir.dt.float32r
BF16 = mybir.dt.bfloat16

MODE = "bf16"            # "bf16" | "f32r"

B, S, D = 2, 2048, 1024
H, DH = 16, 64
NCORES = 8
GROUPS = 4               # head-groups (tensor parallel)
HG = H // GROUPS         # 4 heads per group
FEAT = HG * DH           # 256 features per group
SCALE = 1.0 / 8.0        # 1/sqrt(DH), folded into Wq/bq on host

CHUNK = 512              # seq chunk (PSUM bank = 512 fp32)
NSUB = CHUNK // 128      # 4 seq subtiles per chunk
NCHUNK = S // CHUNK      # 4
KD = D // 128            # 8 k-tiles over D
MT = FEAT // 128         # 2 feature M-tiles per group


def _emit(tc, mode):
    nc = tc.nc
    WDT = BF16 if mode == "bf16" else F32R
    if mode == "bf16":
        x = nc.dram_tensor("x", [S, D], BF16, kind="ExternalInput").ap()
    else:
        x_hi = nc.dram_tensor("x_hi", [S, D], BF16, kind="ExternalInput").ap()
        x_lo = nc.dram_tensor("x_lo", [S, D], BF16, kind="ExternalInput").ap()
    wq = nc.dram_tensor("wq", [D, FEAT], WDT, kind="ExternalInput").ap()
    wk = nc.dram_tensor("wk", [D, FEAT], WDT, kind="ExternalInput").ap()
    wv = nc.dram_tensor("wv", [D, FEAT], WDT, kind="ExternalInput").ap()
    bq = nc.dram_tensor("bq", [FEAT], F32, kind="ExternalInput").ap()
    bk = nc.dram_tensor("bk", [FEAT], F32, kind="ExternalInput").ap()
    bv = nc.dram_tensor("bv", [FEAT], F32, kind="ExternalInput").ap()
    wo = nc.dram_tensor("wo", [FEAT, D], WDT, kind="ExternalInput").ap()
    out = nc.dram_tensor("out", [S, D], F32, kind="ExternalOutput").ap()

    consts = tc.alloc_tile_pool(name="consts", bufs=1)
    weights = tc.alloc_tile_pool(name="weights", bufs=1)
    persist = tc.alloc_tile_pool(name="persist", bufs=1)
    xt_pool = tc.alloc_tile_pool(name="xt", bufs=2)
    et_pool = tc.alloc_tile_pool(name="et", bufs=6)
    rc_pool = tc.alloc_tile_pool(name="rc", bufs=3)
    ob_pool = tc.alloc_tile_pool(name="ob", bufs=2)
    work_ps = tc.alloc_tile_pool(name="work_ps", bufs=6, space="PSUM")
    cx_ps = tc.alloc_tile_pool(name="cx_ps", bufs=2, space="PSUM")

    # constants
    onesf = consts.tile([128, 64], F32)   # f32 scratch (memset can't write f32r)
    nc.vector.memset(onesf, 1.0)
    # tri[k, q] = 1 if q >= k else 0 (f32r memset is unsupported -> f32 there)
    tri = consts.tile([128, 128], BF16 if mode == "bf16" else F32)
    make_upper_triangular(nc, tri, val=1.0, diag=True)

    # weights
    wq_sb = weights.tile([128, KD, MT, 128], WDT)
    nc.sync.dma_start(wq_sb, wq.rearrange("(k p) (m f) -> p k m f", p=128, f=128))
    wk_sb = weights.tile([128, KD, MT, 128], WDT)
    nc.sync.dma_start(wk_sb, wk.rearrange("(k p) (m f) -> p k m f", p=128, f=128))
    wv_sb = weights.tile([128, KD, FEAT], WDT)
    nc.sync.dma_start(wv_sb, wv.rearrange("(k p) f -> p k f", p=128))
    wo_sb = weights.tile([128, MT, D], WDT)
    nc.sync.dma_start(wo_sb, wo.rearrange("(k p) n -> p k n", p=128))
    bqt = weights.tile([128, MT], F32)
    nc.sync.dma_start(bqt, bq.rearrange("(m p) -> p m", p=128))
    bkt = weights.tile([128, MT], F32)
    nc.sync.dma_start(bkt, bk.rearrange("(m p) -> p m", p=128))
    bvb = weights.tile([128, HG, DH], F32)
    nc.sync.dma_start(bvb, bv[None, :].to_broadcast([128, FEAT]).rearrange(
        "p (h f) -> p h f", h=HG))

    # persistent activations
    qt = persist.tile([128, MT, S], WDT)     # Q^T (features on partitions)
    # K^T stored per head in a full 128-partition tile: head h occupies rows
    # 64*(h%2)..+64 (matching qt's packing); the other 64 rows stay ZERO so
    # the scores matmul can contract over K=128 (keeps the full PE array
    # active for HAM) without changing the result.
    kt4 = persist.tile([128, HG, S], WDT)
    nc.vector.memset(kt4, 0.0)
    vaug = persist.tile([128, S // 128, HG, DH + 1], WDT)  # [V_h | 1] per head
    ctxT = persist.tile([128, MT, S], WDT)   # normalized ctx^T
    nc.vector.tensor_copy(vaug[:, :, :, DH],
                          onesf.rearrange("p (a b) -> p a b", a=S // 128))

    def normalize(c, h, cxt):
        """recip(denom), broadcast across partitions on GPSIMD
        (keeps the PE entirely out of the normalization chain), scale ctx."""
        cs = c * CHUNK
        ht, hr = h // 2, 64 * (h % 2)
        rc = rc_pool.tile([1, CHUNK], F32, tag="rc")
        nc.vector.reciprocal(rc, cxt[DH:DH + 1, :])
        bcs = rc_pool.tile([64, CHUNK], F32, tag="bcs")
        nc.gpsimd.partition_broadcast(bcs, rc)
        nc.vector.tensor_mul(ctxT[hr:hr + 64, ht, cs:cs + CHUNK],
                             cxt[0:DH, :], bcs)

    def outproj(c):
        for t in range(NSUB):
            gt = c * NSUB + t
            ob = ob_pool.tile([128, D], F32)
            for n in range(D // 512):
                op = work_ps.tile([128, CHUNK], F32, tag="w")
                for k in range(MT):
                    nc.tensor.matmul(
                        op,
                        ctxT[:, k, gt * 128:(gt + 1) * 128],
                        wo_sb[:, k, 512 * n:512 * (n + 1)],
                        start=(k == 0), stop=(k == MT - 1))
                nc.vector.tensor_copy(ob[:, 512 * n:512 * (n + 1)], op)
            nc.sync.dma_start(out[gt * 128:(gt + 1) * 128, :], ob)

    for c in range(NCHUNK):
        cs = c * CHUNK
        # ---- load x^T chunk via DMA transpose ----
        if mode == "bf16":
            xt = xt_pool.tile([128, KD, CHUNK], BF16, tag="xt")
            for k in range(KD):
                nc.sync.dma_start_transpose(
                    xt[:, k, :], x[cs:cs + CHUNK, 128 * k:128 * (k + 1)])
        else:
            xh = xt_pool.tile([128, KD, CHUNK], BF16, tag="xh")
            xl = xt_pool.tile([128, KD, CHUNK], BF16, tag="xl")
            for k in range(KD):
                nc.sync.dma_start_transpose(
                    xh[:, k, :], x_hi[cs:cs + CHUNK, 128 * k:128 * (k + 1)])
                nc.sync.dma_start_transpose(
                    xl[:, k, :], x_lo[cs:cs + CHUNK, 128 * k:128 * (k + 1)])
            xt = xt_pool.tile([128, KD, CHUNK], F32R, tag="xt")
            for k in range(KD):
                nc.vector.tensor_add(xt[:, k, :], xh[:, k, :], xl[:, k, :])

        # ---- QT / KT projections (features on partitions) ----
        for w_sb, bias_t, dst in ((wq_sb, bqt, qt), (wk_sb, bkt, None)):
            for m in range(MT):
                ps = work_ps.tile([128, CHUNK], F32, tag="w")
                for k in range(KD):
                    nc.tensor.matmul(ps, w_sb[:, k, m, :], xt[:, k, :],
                                     start=(k == 0), stop=(k == KD - 1))
                if dst is not None:
                    nc.scalar.activation(
                        dst[:, m, cs:cs + CHUNK], ps,
                        mybir.ActivationFunctionType.Identity,
                        bias=bias_t[:, m:m + 1], scale=1.0)
                else:
                    for hh in range(2):  # kt4: per-head halves, same rows
                        nc.scalar.activation(
                            kt4[64 * hh:64 * hh + 64, 2 * m + hh, cs:cs + CHUNK],
                            ps[64 * hh:64 * hh + 64, :],
                            mybir.ActivationFunctionType.Identity,
                            bias=bias_t[64 * hh:64 * hh + 64, m:m + 1], scale=1.0)

        # ---- V projection (seq on partitions), augmented with ones col ----
        for t in range(NSUB):
            gt = c * NSUB + t
            ps = work_ps.tile([128, CHUNK], F32, tag="w")
            for k in range(KD):
                nc.tensor.matmul(ps[:, 0:FEAT],
                                 xt[:, k, t * 128:(t + 1) * 128],
                                 wv_sb[:, k, :],
                                 start=(k == 0), stop=(k == KD - 1))
            nc.vector.tensor_add(
                vaug[:, gt, :, 0:DH],
                ps[:, 0:FEAT].rearrange("p (h f) -> p h f", h=HG), bvb)

        # ---- attention for q-chunk c ----
        # Normalizes are deferred two heads (across chunk boundaries) and
        # each chunk's out-projection is deferred into the next chunk, so the
        # recip->broadcast->scale chain never stalls the PE.
        for h in range(HG):
            ht, hr = h // 2, 64 * (h % 2)
            jmax = c * NSUB + NSUB - 1
            cx = cx_ps.tile([DH + 1, CHUNK], F32, tag="cx")
            for j in range(jmax + 1):
                lv = max(0, 128 * j - cs)   # first valid q (chunk-local)
                nq = CHUNK - lv
                sp = work_ps.tile([128, CHUNK], F32, tag="w")
                nc.tensor.matmul(sp[:, 0:nq],
                                 kt4[:, h, 128 * j:128 * (j + 1)],
                                 qt[:, ht, cs + lv:cs + CHUNK])
                et = et_pool.tile([128, CHUNK], WDT)
                nc.scalar.activation(et[:, 0:nq], sp[:, 0:nq],
                                     mybir.ActivationFunctionType.Exp)
                if j >= c * NSUB:  # diagonal block: causal triangular mask
                    nc.vector.tensor_mul(et[:, 0:128], et[:, 0:128], tri)
                nc.tensor.matmul(cx[:, lv:CHUNK], vaug[:, j, h, :],
                                 et[:, 0:nq],
                                 start=(j == 0), stop=(j == jmax))
            normalize(c, h, cx)
            if h == 1 and c > 0:
                outproj(c - 1)

    outproj(NCHUNK - 1)

    for p in (cx_ps, work_ps, ob_pool, rc_pool, et_pool, xt_pool,
              persist, weights, consts):
        p.release()


_BUILT = None


def _build():
    global _BUILT
    if _BUILT is None:
        nc = bacc.Bacc("TRN2", target_bir_lowering=False, debug=False,
                       num_devices=NCORES)
        with tile.TileContext(nc) as tc:
            _emit(tc, MODE)
        nc.compile()
        _BUILT = nc
    return _BUILT


def _bf16(a):
    return np.ascontiguousarray(np.asarray(a, dtype=np.float32)).astype(
        ml_dtypes.bfloat16)


def _f32(a):
    return np.ascontiguousarray(np.asarray(a, dtype=np.float32))


def _shards(inputs):
    x = np.asarray(inputs["x"], dtype=np.float32)
    maps = []
    for core in range(NCORES):
        b, g = core // GROUPS, core % GROUPS
        f0 = g * FEAT
        m = {
            "bq": _f32(np.asarray(inputs["bq"], np.float32)[f0:f0 + FEAT] * SCALE),
            "bk": _f32(np.asarray(inputs["bk"], np.float32)[f0:f0 + FEAT]),
            "bv": _f32(np.asarray(inputs["bv"], np.float32)[f0:f0 + FEAT]),
        }
        wq_s = np.asarray(inputs["Wq"], np.float32)[:, f0:f0 + FEAT] * SCALE
        wk_s = np.asarray(inputs["Wk"], np.float32)[:, f0:f0 + FEAT]
        wv_s = np.asarray(inputs["Wv"], np.float32)[:, f0:f0 + FEAT]
        wo_s = np.asarray(inputs["Wo"], np.float32)[f0:f0 + FEAT, :]
        if MODE == "bf16":
            m["x"] = _bf16(x[b])
            m["wq"], m["wk"], m["wv"], m["wo"] = (
                _bf16(wq_s), _bf16(wk_s), _bf16(wv_s), _bf16(wo_s))
        else:
            hi = x[b].astype(ml_dtypes.bfloat16)
            lo = (x[b] - hi.astype(np.float32)).astype(ml_dtypes.bfloat16)
            m["x_hi"] = np.ascontiguousarray(hi)
            m["x_lo"] = np.ascontiguousarray(lo)
            m["wq"], m["wk"], m["wv"], m["wo"] = (
                _f32(wq_s), _f32(wk_s), _f32(wv_s), _f32(wo_s))
        maps.append(m)
    return maps


def kernel(trace=False, **inputs):
    nc = _build()
    res = run_bass_kernel_spmd(nc, _shards(inputs), core_ids=list(range(NCORES)),
                               trace=trace)
    partial = np.stack([r_["out"] for r_ in res.results])  # [8, S, D]
    acc = partial.reshape(B, GROUPS, S, D).astype(np.float64).sum(axis=1)
    acc += np.asarray(inputs["bo"], dtype=np.float64)
    out = acc.astype(np.float32)
    if trace:
        return out, res
    return out


# revision 19
# speedup vs baseline: 1.4333x; 1.0161x over previous
# Multi-head causal self-attention (B=2, S=2048, D=1024, H=16, Dh=64) on 8
# Trainium2 NeuronCores.
#
# Sharding: core i -> (batch b = i // 4, head-group g = i % 4). Each core
# computes attention for its batch's 4 heads (feature columns 256g:256g+256 of
# the QKV projections, rows 256g:256g+256 of Wo) and produces a partial
# out-projection [S, D]. Host sums the 4 partials per batch and adds bo.
#
# MODE selects matmul operand precision:
#   "bf16": operands bf16 (fp32 PSUM accumulation), x^T loaded straight from
#           DRAM via 2-byte DMA-transpose. Fastest; ~3e-3 rel error.
#   "f32r": operands float32r (PE keeps more mantissa, 1.5 cycles/row). x is
#           shipped as a bf16 hi/lo pair, DMA-transposed, and recombined on
#           DVE into f32r x^T. ~3e-4 rel error.
#
# Per-core dataflow:
#   1. xT chunk [D, 512] via DMA-transpose.
#   2. QT = Wq_s^T xT + bq [256, S] (features on partitions), same for KT.
#      V = xT^T Wv_s + bv  [S, 256] (seq on partitions), stored augmented with
#      a ones column per head ([V_h | 1]) so the attention matmul also
#      accumulates the softmax denominator.
#   3. per (head, q-chunk): S^T tile = K_h Q_h^T [k, q]; E = exp(S^T) (scores
#      pre-scaled by 1/sqrt(Dh) via host-side Wq scaling; magnitudes are small
#      enough that max-subtraction is unnecessary); causality = skip k>q tiles
#      + triangular mask multiply on diagonal blocks; [ctx^T; denom] +=
#      [V_h | 1]^T E.
#   4. normalize (deferred two heads to keep PE fed): recip(denom) via the
#      fast Newton-Raphson DVE op, broadcast across partitions via K=1
#      matmul, multiply.
#   5. out_partial = ctxT^T Wo_s, DMA out.

import numpy as np
import ml_dtypes

import concourse.bass as bass
import concourse.mybir as mybir
import concourse.tile as tile
from concourse import bacc
from concourse.bass_utils import run_bass_kernel_spmd
from concourse.masks import make_upper_triangular

F32 = mybir.dt.float32
F32R = mybir.dt.float32r
BF16 = mybir.dt.bfloat16

MODE = "bf16"            # "bf16" | "f32r"

B, S, D = 2, 2048, 1024
H, DH = 16, 64
NCORES = 8
GROUPS = 4               # head-groups (tensor parallel)
HG = H // GROUPS         # 4 heads per group
FEAT = HG * DH           # 256 features per group
SCALE = 1.0 / 8.0        # 1/sqrt(DH), folded into Wq/bq on host

CHUNK = 512              # seq chunk (PSUM bank = 512 fp32)
NSUB = CHUNK // 128      # 4 seq subtiles per chunk
NCHUNK = S // CHUNK      # 4
KD = D // 128            # 8 k-tiles over D
MT = FEAT // 128         # 2 feature M-tiles per group


def _emit(tc, mode):
    nc = tc.nc
    WDT = BF16 if mode == "bf16" else F32R
    if mode == "bf16":
        x = nc.dram_tensor("x", [S, D], BF16, kind="ExternalInput").ap()
    else:
        x_hi = nc.dram_tensor("x_hi", [S, D], BF16, kind="ExternalInput").ap()
        x_lo = nc.dram_tensor("x_lo", [S, D], BF16, kind="ExternalInput").ap()
    wq = nc.dram_tensor("wq", [D, FEAT], WDT, kind="ExternalInput").ap()
    wk = nc.dram_tensor("wk", [D, FEAT], WDT, kind="ExternalInput").ap()
    wv = nc.dram_tensor("wv", [D, FEAT], WDT, kind="ExternalInput").ap()
    bq = nc.dram_tensor("bq", [FEAT], F32, kind="ExternalInput").ap()
    bk = nc.dram_tensor("bk", [FEAT], F32, kind="ExternalInput").ap()
    bv = nc.dram_tensor("bv", [FEAT], F32, kind="ExternalInput").ap()
    wo = nc.dram_tensor("wo", [FEAT, D], WDT, kind="ExternalInput").ap()
    out = nc.dram_tensor("out", [S, D], F32, kind="ExternalOutput").ap()

    consts = tc.alloc_tile_pool(name="consts", bufs=1)
    weights = tc.alloc_tile_pool(name="weights", bufs=1)
    persist = tc.alloc_tile_pool(name="persist", bufs=1)
    xt_pool = tc.alloc_tile_pool(name="xt", bufs=2)
    et_pool = tc.alloc_tile_pool(name="et", bufs=6)
    rc_pool = tc.alloc_tile_pool(name="rc", bufs=3)
    ob_pool = tc.alloc_tile_pool(name="ob", bufs=2)
    work_ps = tc.alloc_tile_pool(name="work_ps", bufs=6, space="PSUM")
    cx_ps = tc.alloc_tile_pool(name="cx_ps", bufs=2, space="PSUM")

    # constants
    onesf = consts.tile([128, 64], F32)   # f32 scratch (memset can't write f32r)
    nc.vector.memset(onesf, 1.0)
    # tri[k, q] = 1 if q >= k else 0 (f32r memset is unsupported -> f32 there)
    tri = consts.tile([128, 128], BF16 if mode == "bf16" else F32)
    make_upper_triangular(nc, tri, val=1.0, diag=True)

    # weights
    wq_sb = weights.tile([128, KD, MT, 128], WDT)
    nc.sync.dma_start(wq_sb, wq.rearrange("(k p) (m f) -> p k m f", p=128, f=128))
    wk_sb = weights.tile([128, KD, MT, 128], WDT)
    nc.sync.dma_start(wk_sb, wk.rearrange("(k p) (m f) -> p k m f", p=128, f=128))
    wv_sb = weights.tile([128, KD, FEAT], WDT)
    nc.sync.dma_start(wv_sb, wv.rearrange("(k p) f -> p k f", p=128))
    wo_sb = weights.tile([128, MT, D], WDT)
    nc.sync.dma_start(wo_sb, wo.rearrange("(k p) n -> p k n", p=128))
    bqt = weights.tile([128, MT], F32)
    nc.sync.dma_start(bqt, bq.rearrange("(m p) -> p m", p=128))
    bkt = weights.tile([128, MT], F32)
    nc.sync.dma_start(bkt, bk.rearrange("(m p) -> p m", p=128))
    bvb = weights.tile([128, HG, DH], F32)
    nc.sync.dma_start(bvb, bv[None, :].to_broadcast([128, FEAT]).rearrange(
        "p (h f) -> p h f", h=HG))

    # persistent activations
    qt = persist.tile([128, MT, S], WDT)     # Q^T (features on partitions)
    # K^T stored per head in a full 128-partition tile: head h occupies rows
    # 64*(h%2)..+64 (matching qt's packing); the other 64 rows stay ZERO so
    # the scores matmul can contract over K=128 (keeps the full PE array
    # active for HAM) without changing the result.
    kt4 = persist.tile([128, HG, S], WDT)
    nc.vector.memset(kt4, 0.0)
    vaug = persist.tile([128, S // 128, HG, DH + 1], WDT)  # [V_h | 1] per head
    ctxT = persist.tile([128, MT, S], WDT)   # normalized ctx^T
    nc.vector.tensor_copy(vaug[:, :, :, DH],
                          onesf.rearrange("p (a b) -> p a b", a=S // 128))

    def normalize(c, h, cxt):
        """recip(denom), broadcast across partitions on GPSIMD
        (keeps the PE entirely out of the normalization chain), scale ctx."""
        cs = c * CHUNK
        ht, hr = h // 2, 64 * (h % 2)
        rc = rc_pool.tile([1, CHUNK], F32, tag="rc")
        nc.vector.reciprocal(rc, cxt[DH:DH + 1, :])
        bcs = rc_pool.tile([64, CHUNK], F32, tag="bcs")
        nc.gpsimd.partition_broadcast(bcs, rc)
        nc.vector.tensor_mul(ctxT[hr:hr + 64, ht, cs:cs + CHUNK],
                             cxt[0:DH, :], bcs)

    def outproj(c):
        for t in range(NSUB):
            gt = c * NSUB + t
            ob = ob_pool.tile([128, D], F32)
            for n in range(D // 512):
                op = work_ps.tile([128, CHUNK], F32, tag="w")
                for k in range(MT):
                    nc.tensor.matmul(
                        op,
                        ctxT[:, k, gt * 128:(gt + 1) * 128],
                        wo_sb[:, k, 512 * n:512 * (n + 1)],
                        start=(k == 0), stop=(k == MT - 1))
                nc.vector.tensor_copy(ob[:, 512 * n:512 * (n + 1)], op)
            nc.sync.dma_start(out[gt * 128:(gt + 1) * 128, :], ob)

    for c in range(NCHUNK):
        cs = c * CHUNK
        # ---- load x^T chunk via DMA transpose ----
        if mode == "bf16":
            xt = xt_pool.tile([128, KD, CHUNK], BF16, tag="xt")
            for k in range(KD):
                nc.sync.dma_start_transpose(
                    xt[:, k, :], x[cs:cs + CHUNK, 128 * k:128 * (k + 1)])
        else:
            xh = xt_pool.tile([128, KD, CHUNK], BF16, tag="xh")
            xl = xt_pool.tile([128, KD, CHUNK], BF16, tag="xl")
            for k in range(KD):
                nc.sync.dma_start_transpose(
                    xh[:, k, :], x_hi[cs:cs + CHUNK, 128 * k:128 * (k + 1)])
                nc.sync.dma_start_transpose(
                    xl[:, k, :], x_lo[cs:cs + CHUNK, 128 * k:128 * (k + 1)])
            xt = xt_pool.tile([128, KD, CHUNK], F32R, tag="xt")
            for k in range(KD):
                nc.vector.tensor_add(xt[:, k, :], xh[:, k, :], xl[:, k, :])

        def proj_qkt(w_sb, bias_t, dst):
            for m in range(MT):
                ps = work_ps.tile([128, CHUNK], F32, tag="w", name="ps")
                for k in range(KD):
                    nc.tensor.matmul(ps, w_sb[:, k, m, :], xt[:, k, :],
                                     start=(k == 0), stop=(k == KD - 1))
                if dst is not None:
                    nc.scalar.activation(
                        dst[:, m, cs:cs + CHUNK], ps,
                        mybir.ActivationFunctionType.Identity,
                        bias=bias_t[:, m:m + 1], scale=1.0)
                else:
                    for hh in range(2):  # kt4: per-head halves, same rows
                        nc.scalar.activation(
                            kt4[64 * hh:64 * hh + 64, 2 * m + hh, cs:cs + CHUNK],
                            ps[64 * hh:64 * hh + 64, :],
                            mybir.ActivationFunctionType.Identity,
                            bias=bias_t[64 * hh:64 * hh + 64, m:m + 1], scale=1.0)

        def proj_v():
            # V projection (seq on partitions), augmented with ones col
            for t in range(NSUB):
                gt = c * NSUB + t
                ps = work_ps.tile([128, CHUNK], F32, tag="w", name="ps")
                for k in range(KD):
                    nc.tensor.matmul(ps[:, 0:FEAT],
                                     xt[:, k, t * 128:(t + 1) * 128],
                                     wv_sb[:, k, :],
                                     start=(k == 0), stop=(k == KD - 1))
                nc.vector.tensor_add(
                    vaug[:, gt, :, 0:DH],
                    ps[:, 0:FEAT].rearrange("p (h f) -> p h f", h=HG), bvb)

        jmax = c * NSUB + NSUB - 1

        def attn(h, cx, j0, j1, first, last):
            ht = h // 2
            for j in range(j0, j1):
                lv = max(0, 128 * j - cs)   # first valid q (chunk-local)
                nq = CHUNK - lv
                sp = work_ps.tile([128, CHUNK], F32, tag="w", name="sp")
                nc.tensor.matmul(sp[:, 0:nq],
                                 kt4[:, h, 128 * j:128 * (j + 1)],
                                 qt[:, ht, cs + lv:cs + CHUNK])
                et = et_pool.tile([128, CHUNK], WDT, name="et")
                nc.scalar.activation(et[:, 0:nq], sp[:, 0:nq],
                                     mybir.ActivationFunctionType.Exp)
                if j >= c * NSUB:  # diagonal block: causal triangular mask
                    nc.vector.tensor_mul(et[:, 0:128], et[:, 0:128], tri)
                nc.tensor.matmul(cx[:, lv:CHUNK], vaug[:, j, h, :],
                                 et[:, 0:nq],
                                 start=(first and j == j0),
                                 stop=(last and j == j1 - 1),
                                 skip_group_check=True)

        # Emission order: QT first, then head 0's off-diagonal attention (it
        # only needs qt + prior chunks' kt/v), with KT/V projections emitted
        # in between so the chunk-start exp never stalls the PE; normalizes
        # are PE-free, and each chunk's out-projection is deferred into the
        # next chunk.
        proj_qkt(wq_sb, bqt, qt)
        cx0 = cx_ps.tile([DH + 1, CHUNK], F32, tag="cx", name="cx0")
        attn(0, cx0, 0, c * NSUB, True, False)
        proj_qkt(wk_sb, bkt, None)
        proj_v()
        attn(0, cx0, c * NSUB, jmax + 1, c == 0, True)
        normalize(c, 0, cx0)
        for h in range(1, HG):
            cx = cx_ps.tile([DH + 1, CHUNK], F32, tag="cx", name="cx")
            attn(h, cx, 0, jmax + 1, True, True)
            normalize(c, h, cx)
            if h == 1 and c > 0:
                outproj(c - 1)

    outproj(NCHUNK - 1)

    for p in (cx_ps, work_ps, ob_pool, rc_pool, et_pool, xt_pool,
              persist, weights, consts):
        p.release()


_BUILT = None


def _build():
    global _BUILT
    if _BUILT is None:
        nc = bacc.Bacc("TRN2", target_bir_lowering=False, debug=False,
                       num_devices=NCORES)
        with tile.TileContext(nc) as tc:
            _emit(tc, MODE)
        nc.compile()
        _BUILT = nc
    return _BUILT


def _bf16(a):
    return np.ascontiguousarray(np.asarray(a, dtype=np.float32)).astype(
        ml_dtypes.bfloat16)


def _f32(a):
    return np.ascontiguousarray(np.asarray(a, dtype=np.float32))


def _shards(inputs):
    x = np.asarray(inputs["x"], dtype=np.float32)
    maps = []
    for core in range(NCORES):
        b, g = core // GROUPS, core % GROUPS
        f0 = g * FEAT
        m = {
            "bq": _f32(np.asarray(inputs["bq"], np.float32)[f0:f0 + FEAT] * SCALE),
            "bk": _f32(np.asarray(inputs["bk"], np.float32)[f0:f0 + FEAT]),
            "bv": _f32(np.asarray(inputs["bv"], np.float32)[f0:f0 + FEAT]),
        }
        wq_s = np.asarray(inputs["Wq"], np.float32)[:, f0:f0 + FEAT] * SCALE
        wk_s = np.asarray(inputs["Wk"], np.float32)[:, f0:f0 + FEAT]
        wv_s = np.asarray(inputs["Wv"], np.float32)[:, f0:f0 + FEAT]
        wo_s = np.asarray(inputs["Wo"], np.float32)[f0:f0 + FEAT, :]
        if MODE == "bf16":
            m["x"] = _bf16(x[b])
            m["wq"], m["wk"], m["wv"], m["wo"] = (
                _bf16(wq_s), _bf16(wk_s), _bf16(wv_s), _bf16(wo_s))
        else:
            hi = x[b].astype(ml_dtypes.bfloat16)
            lo = (x[b] - hi.astype(np.float32)).astype(ml_dtypes.bfloat16)
            m["x_hi"] = np.ascontiguousarray(hi)
            m["x_lo"] = np.ascontiguousarray(lo)
            m["wq"], m["wk"], m["wv"], m["wo"] = (
                _f32(wq_s), _f32(wk_s), _f32(wv_s), _f32(wo_s))
        maps.append(m)
    return maps


def kernel(trace=False, **inputs):
    nc = _build()
    res = run_bass_kernel_spmd(nc, _shards(inputs), core_ids=list(range(NCORES)),
                               trace=trace)
    partial = np.stack([r_["out"] for r_ in res.results])  # [8, S, D]
    acc = partial.reshape(B, GROUPS, S, D).astype(np.float64).sum(axis=1)
    acc += np.asarray(inputs["bo"], dtype=np.float64)
    out = acc.astype(np.float32)
    if trace:
        return out, res
    return out


# revision 21
# speedup vs baseline: 1.5855x; 1.1062x over previous
# Multi-head causal self-attention (B=2, S=2048, D=1024, H=16, Dh=64) on 8
# Trainium2 NeuronCores.
#
# Sharding: core i -> (batch b = i // 4, head-group g = i % 4). Each core
# computes attention for its batch's 4 heads (feature columns 256g:256g+256 of
# the QKV projections, rows 256g:256g+256 of Wo) and produces a partial
# out-projection [S, D]. Host sums the 4 partials per batch and adds bo.
#
# MODE selects matmul operand precision:
#   "bf16": operands bf16 (fp32 PSUM accumulation), x^T loaded straight from
#           DRAM via 2-byte DMA-transpose. Fastest; ~3e-3 rel error.
#   "f32r": operands float32r (PE keeps more mantissa, 1.5 cycles/row). x is
#           shipped as a bf16 hi/lo pair, DMA-transposed, and recombined on
#           DVE into f32r x^T. ~3e-4 rel error.
#
# Per-core dataflow:
#   1. xT chunk [D, 512] via DMA-transpose.
#   2. QT = Wq_s^T xT + bq [256, S] (features on partitions), same for KT.
#      V = xT^T Wv_s + bv  [S, 256] (seq on partitions), stored augmented with
#      a ones column per head ([V_h | 1]) so the attention matmul also
#      accumulates the softmax denominator.
#   3. per (head, q-chunk): S^T tile = K_h Q_h^T [k, q]; E = exp(S^T) (scores
#      pre-scaled by 1/sqrt(Dh) via host-side Wq scaling; magnitudes are small
#      enough that max-subtraction is unnecessary); causality = skip k>q tiles
#      + triangular mask multiply on diagonal blocks; [ctx^T; denom] +=
#      [V_h | 1]^T E.
#   4. normalize (deferred two heads to keep PE fed): recip(denom) via the
#      fast Newton-Raphson DVE op, broadcast across partitions via K=1
#      matmul, multiply.
#   5. out_partial = ctxT^T Wo_s, DMA out.

import numpy as np
import ml_dtypes

import concourse.bass as bass
import concourse.mybir as mybir
import concourse.tile as tile
from concourse import bacc
from concourse.bass_utils import run_bass_kernel_spmd
from concourse.masks import make_upper_triangular

F32 = mybir.dt.float32
F32R = mybir.dt.float32r
BF16 = mybir.dt.bfloat16

MODE = "bf16"            # "bf16" | "f32r"

B, S, D = 2, 2048, 1024
H, DH = 16, 64
NCORES = 8
GROUPS = 4               # head-groups (tensor parallel)
HG = H // GROUPS         # 4 heads per group
FEAT = HG * DH           # 256 features per group
SCALE = 1.0 / 8.0        # 1/sqrt(DH), folded into Wq/bq on host

CHUNK = 512              # seq chunk (PSUM bank = 512 fp32)
NSUB = CHUNK // 128      # 4 seq subtiles per chunk
NCHUNK = S // CHUNK      # 4
KD = D // 128            # 8 k-tiles over D
MT = FEAT // 128         # 2 feature M-tiles per group


def _emit(tc, mode):
    nc = tc.nc
    WDT = BF16 if mode == "bf16" else F32R
    if mode == "bf16":
        x = nc.dram_tensor("x", [S, D], BF16, kind="ExternalInput").ap()
    else:
        x_hi = nc.dram_tensor("x_hi", [S, D], BF16, kind="ExternalInput").ap()
        x_lo = nc.dram_tensor("x_lo", [S, D], BF16, kind="ExternalInput").ap()
    wq = nc.dram_tensor("wq", [D, FEAT], WDT, kind="ExternalInput").ap()
    wk = nc.dram_tensor("wk", [D, FEAT], WDT, kind="ExternalInput").ap()
    wv = nc.dram_tensor("wv", [D, FEAT], WDT, kind="ExternalInput").ap()
    bq = nc.dram_tensor("bq", [FEAT], F32, kind="ExternalInput").ap()
    bk = nc.dram_tensor("bk", [FEAT], F32, kind="ExternalInput").ap()
    bv = nc.dram_tensor("bv", [FEAT], F32, kind="ExternalInput").ap()
    wo = nc.dram_tensor("wo", [FEAT, D], WDT, kind="ExternalInput").ap()
    out = nc.dram_tensor("out", [S, D], F32, kind="ExternalOutput").ap()

    consts = tc.alloc_tile_pool(name="consts", bufs=1)
    weights = tc.alloc_tile_pool(name="weights", bufs=1)
    persist = tc.alloc_tile_pool(name="persist", bufs=1)
    xt_pool = tc.alloc_tile_pool(name="xt", bufs=2)
    et_pool = tc.alloc_tile_pool(name="et", bufs=6)
    rc_pool = tc.alloc_tile_pool(name="rc", bufs=3)
    ob_pool = tc.alloc_tile_pool(name="ob", bufs=2)
    work_ps = tc.alloc_tile_pool(name="work_ps", bufs=6, space="PSUM")
    cx_ps = tc.alloc_tile_pool(name="cx_ps", bufs=2, space="PSUM")

    # constants
    onesf = consts.tile([128, 64], F32)   # f32 scratch (memset can't write f32r)
    nc.vector.memset(onesf, 1.0)
    # tri[k, q] = 1 if q >= k else 0 (f32r memset is unsupported -> f32 there)
    tri = consts.tile([128, 128], BF16 if mode == "bf16" else F32)
    make_upper_triangular(nc, tri, val=1.0, diag=True)

    # weights
    wq_sb = weights.tile([128, KD, MT, 128], WDT)
    nc.sync.dma_start(wq_sb, wq.rearrange("(k p) (m f) -> p k m f", p=128, f=128))
    wk_sb = weights.tile([128, KD, MT, 128], WDT)
    nc.sync.dma_start(wk_sb, wk.rearrange("(k p) (m f) -> p k m f", p=128, f=128))
    wv_sb = weights.tile([128, KD, FEAT], WDT)
    nc.sync.dma_start(wv_sb, wv.rearrange("(k p) f -> p k f", p=128))
    wo_sb = weights.tile([128, MT, D], WDT)
    nc.sync.dma_start(wo_sb, wo.rearrange("(k p) n -> p k n", p=128))
    bqt = weights.tile([128, MT], F32)
    nc.sync.dma_start(bqt, bq.rearrange("(m p) -> p m", p=128))
    bkt = weights.tile([128, MT], F32)
    nc.sync.dma_start(bkt, bk.rearrange("(m p) -> p m", p=128))
    bvb = weights.tile([128, HG, DH], F32)
    nc.sync.dma_start(bvb, bv[None, :].to_broadcast([128, FEAT]).rearrange(
        "p (h f) -> p h f", h=HG))

    # persistent activations
    qt = persist.tile([128, MT, S], WDT)     # Q^T (features on partitions)
    # K^T stored per head in a full 128-partition tile: head h occupies rows
    # 64*(h%2)..+64 (matching qt's packing); the other 64 rows stay ZERO so
    # the scores matmul can contract over K=128 (keeps the full PE array
    # active for HAM) without changing the result.
    kt4 = persist.tile([128, HG, S], WDT)
    nc.vector.memset(kt4, 0.0)
    vaug = persist.tile([128, S // 128, HG, DH + 1], WDT)  # [V_h | 1] per head
    ctxT = persist.tile([128, MT, S], WDT)   # normalized ctx^T
    nc.vector.tensor_copy(vaug[:, :, :, DH],
                          onesf.rearrange("p (a b) -> p a b", a=S // 128))

    def normalize(c, h, cxt):
        """recip(denom), broadcast across partitions on GPSIMD
        (keeps the PE entirely out of the normalization chain), scale ctx."""
        cs = c * CHUNK
        ht, hr = h // 2, 64 * (h % 2)
        rc0 = rc_pool.tile([1, CHUNK], F32, tag="rc0")
        nc.vector.tensor_copy(rc0, cxt[DH:DH + 1, :])
        rc = rc_pool.tile([1, CHUNK], F32, tag="rc")
        nc.vector.reciprocal_approx_fast(rc, rc0)
        bcs = rc_pool.tile([64, CHUNK], F32, tag="bcs")
        nc.gpsimd.partition_broadcast(bcs, rc)
        nc.vector.tensor_mul(ctxT[hr:hr + 64, ht, cs:cs + CHUNK],
                             cxt[0:DH, :], bcs)

    def outproj(c):
        for t in range(NSUB):
            gt = c * NSUB + t
            ob = ob_pool.tile([128, D], F32)
            for n in range(D // 512):
                op = work_ps.tile([128, CHUNK], F32, tag="w")
                for k in range(MT):
                    nc.tensor.matmul(
                        op,
                        ctxT[:, k, gt * 128:(gt + 1) * 128],
                        wo_sb[:, k, 512 * n:512 * (n + 1)],
                        start=(k == 0), stop=(k == MT - 1))
                nc.vector.tensor_copy(ob[:, 512 * n:512 * (n + 1)], op)
            nc.sync.dma_start(out[gt * 128:(gt + 1) * 128, :], ob)

    for c in range(NCHUNK):
        cs = c * CHUNK
        # ---- load x^T chunk via DMA transpose ----
        if mode == "bf16":
            xt = xt_pool.tile([128, KD, CHUNK], BF16, tag="xt")
            for k in range(KD):
                nc.sync.dma_start_transpose(
                    xt[:, k, :], x[cs:cs + CHUNK, 128 * k:128 * (k + 1)])
        else:
            xh = xt_pool.tile([128, KD, CHUNK], BF16, tag="xh")
            xl = xt_pool.tile([128, KD, CHUNK], BF16, tag="xl")
            for k in range(KD):
                nc.sync.dma_start_transpose(
                    xh[:, k, :], x_hi[cs:cs + CHUNK, 128 * k:128 * (k + 1)])
                nc.sync.dma_start_transpose(
                    xl[:, k, :], x_lo[cs:cs + CHUNK, 128 * k:128 * (k + 1)])
            xt = xt_pool.tile([128, KD, CHUNK], F32R, tag="xt")
            for k in range(KD):
                nc.vector.tensor_add(xt[:, k, :], xh[:, k, :], xl[:, k, :])

        def proj_qkt(w_sb, bias_t, dst):
            for m in range(MT):
                ps = work_ps.tile([128, CHUNK], F32, tag="w", name="ps")
                for k in range(KD):
                    nc.tensor.matmul(ps, w_sb[:, k, m, :], xt[:, k, :],
                                     start=(k == 0), stop=(k == KD - 1))
                if dst is not None:
                    nc.scalar.activation(
                        dst[:, m, cs:cs + CHUNK], ps,
                        mybir.ActivationFunctionType.Identity,
                        bias=bias_t[:, m:m + 1], scale=1.0)
                else:
                    for hh in range(2):  # kt4: per-head halves, same rows
                        nc.scalar.activation(
                            kt4[64 * hh:64 * hh + 64, 2 * m + hh, cs:cs + CHUNK],
                            ps[64 * hh:64 * hh + 64, :],
                            mybir.ActivationFunctionType.Identity,
                            bias=bias_t[64 * hh:64 * hh + 64, m:m + 1], scale=1.0)

        def proj_v():
            # V projection (seq on partitions), augmented with ones col
            for t in range(NSUB):
                gt = c * NSUB + t
                ps = work_ps.tile([128, CHUNK], F32, tag="w", name="ps")
                for k in range(KD):
                    nc.tensor.matmul(ps[:, 0:FEAT],
                                     xt[:, k, t * 128:(t + 1) * 128],
                                     wv_sb[:, k, :],
                                     start=(k == 0), stop=(k == KD - 1))
                nc.vector.tensor_add(
                    vaug[:, gt, :, 0:DH],
                    ps[:, 0:FEAT].rearrange("p (h f) -> p h f", h=HG), bvb)

        jmax = c * NSUB + NSUB - 1

        def attn(h, cx, j0, j1, first, last):
            ht = h // 2
            for j in range(j0, j1):
                lv = max(0, 128 * j - cs)   # first valid q (chunk-local)
                nq = CHUNK - lv
                sp = work_ps.tile([128, CHUNK], F32, tag="w", name="sp")
                nc.tensor.matmul(sp[:, 0:nq],
                                 kt4[:, h, 128 * j:128 * (j + 1)],
                                 qt[:, ht, cs + lv:cs + CHUNK])
                et = et_pool.tile([128, CHUNK], WDT, name="et")
                nc.scalar.activation(et[:, 0:nq], sp[:, 0:nq],
                                     mybir.ActivationFunctionType.Exp)
                if j >= c * NSUB:  # diagonal block: causal triangular mask
                    nc.vector.tensor_mul(et[:, 0:128], et[:, 0:128], tri)
                nc.tensor.matmul(cx[:, lv:CHUNK], vaug[:, j, h, :],
                                 et[:, 0:nq],
                                 start=(first and j == j0),
                                 stop=(last and j == j1 - 1),
                                 skip_group_check=True)

        # Emission order: QT first, then head 0's off-diagonal attention (it
        # only needs qt + prior chunks' kt/v), with KT/V projections emitted
        # in between so the chunk-start exp never stalls the PE; normalizes
        # are PE-free, and each chunk's out-projection is deferred into the
        # next chunk.
        proj_qkt(wq_sb, bqt, qt)
        cx0 = cx_ps.tile([DH + 1, CHUNK], F32, tag="cx", name="cx0")
        attn(0, cx0, 0, c * NSUB, True, False)
        proj_qkt(wk_sb, bkt, None)
        proj_v()
        attn(0, cx0, c * NSUB, jmax + 1, c == 0, True)
        normalize(c, 0, cx0)
        for h in range(1, HG):
            cx = cx_ps.tile([DH + 1, CHUNK], F32, tag="cx", name="cx")
            attn(h, cx, 0, jmax + 1, True, True)
            normalize(c, h, cx)
            if h == 1 and c > 0:
                outproj(c - 1)

    outproj(NCHUNK - 1)

    for p in (cx_ps, work_ps, ob_pool, rc_pool, et_pool, xt_pool,
              persist, weights, consts):
        p.release()


_BUILT = None


def _build():
    global _BUILT
    if _BUILT is None:
        nc = bacc.Bacc("TRN2", target_bir_lowering=False, debug=False,
                       num_devices=NCORES)
        with tile.TileContext(nc) as tc:
            _emit(tc, MODE)
        nc.compile()
        _BUILT = nc
    return _BUILT


def _bf16(a):
    return np.ascontiguousarray(np.asarray(a, dtype=np.float32)).astype(
        ml_dtypes.bfloat16)


def _f32(a):
    return np.ascontiguousarray(np.asarray(a, dtype=np.float32))


def _shards(inputs):
    x = np.asarray(inputs["x"], dtype=np.float32)
    maps = []
    for core in range(NCORES):
        b, g = core // GROUPS, core % GROUPS
        f0 = g * FEAT
        m = {
            "bq": _f32(np.asarray(inputs["bq"], np.float32)[f0:f0 + FEAT] * SCALE),
            "bk": _f32(np.asarray(inputs["bk"], np.float32)[f0:f0 + FEAT]),
            "bv": _f32(np.asarray(inputs["bv"], np.float32)[f0:f0 + FEAT]),
        }
        wq_s = np.asarray(inputs["Wq"], np.float32)[:, f0:f0 + FEAT] * SCALE
        wk_s = np.asarray(inputs["Wk"], np.float32)[:, f0:f0 + FEAT]
        wv_s = np.asarray(inputs["Wv"], np.float32)[:, f0:f0 + FEAT]
        wo_s = np.asarray(inputs["Wo"], np.float32)[f0:f0 + FEAT, :]
        if MODE == "bf16":
            m["x"] = _bf16(x[b])
            m["wq"], m["wk"], m["wv"], m["wo"] = (
                _bf16(wq_s), _bf16(wk_s), _bf16(wv_s), _bf16(wo_s))
        else:
            hi = x[b].astype(ml_dtypes.bfloat16)
            lo = (x[b] - hi.astype(np.float32)).astype(ml_dtypes.bfloat16)
            m["x_hi"] = np.ascontiguousarray(hi)
            m["x_lo"] = np.ascontiguousarray(lo)
            m["wq"], m["wk"], m["wv"], m["wo"] = (
                _f32(wq_s), _f32(wk_s), _f32(wv_s), _f32(wo_s))
        maps.append(m)
    return maps


def kernel(trace=False, **inputs):
    nc = _build()
    res = run_bass_kernel_spmd(nc, _shards(inputs), core_ids=list(range(NCORES)),
                               trace=trace)
    partial = np.stack([r_["out"] for r_ in res.results])  # [8, S, D]
    acc = partial.reshape(B, GROUPS, S, D).astype(np.float64).sum(axis=1)
    acc += np.asarray(inputs["bo"], dtype=np.float64)
    out = acc.astype(np.float32)
    if trace:
        return out, res
    return out


# revision 24
# speedup vs baseline: 1.6270x; 1.0262x over previous
# Multi-head causal self-attention (B=2, S=2048, D=1024, H=16, Dh=64) on 8
# Trainium2 NeuronCores.
#
# Sharding: core i -> (batch b = i // 4, head-group g = i % 4). Each core
# computes attention for its batch's 4 heads (feature columns 256g:256g+256 of
# the QKV projections, rows 256g:256g+256 of Wo) and produces a partial
# out-projection [S, D]. Host sums the 4 partials per batch and adds bo.
#
# MODE selects matmul operand precision:
#   "bf16": operands bf16 (fp32 PSUM accumulation), x^T loaded straight from
#           DRAM via 2-byte DMA-transpose. Fastest; ~3e-3 rel error.
#   "f32r": operands float32r (PE keeps more mantissa, 1.5 cycles/row). x is
#           shipped as a bf16 hi/lo pair, DMA-transposed, and recombined on
#           DVE into f32r x^T. ~3e-4 rel error.
#
# Per-core dataflow:
#   1. xT chunk [D, 512] via DMA-transpose.
#   2. QT = Wq_s^T xT + bq [256, S] (features on partitions), same for KT.
#      V = xT^T Wv_s + bv  [S, 256] (seq on partitions), stored augmented with
#      a ones column per head ([V_h | 1]) so the attention matmul also
#      accumulates the softmax denominator.
#   3. per (head, q-chunk): S^T tile = K_h Q_h^T [k, q]; E = exp(S^T) (scores
#      pre-scaled by 1/sqrt(Dh) via host-side Wq scaling; magnitudes are small
#      enough that max-subtraction is unnecessary); causality = skip k>q tiles
#      + triangular mask multiply on diagonal blocks; [ctx^T; denom] +=
#      [V_h | 1]^T E.
#   4. normalize (deferred two heads to keep PE fed): recip(denom) via the
#      fast Newton-Raphson DVE op, broadcast across partitions via K=1
#      matmul, multiply.
#   5. out_partial = ctxT^T Wo_s, DMA out.

import numpy as np
import ml_dtypes

import concourse.bass as bass
import concourse.mybir as mybir
import concourse.tile as tile
from concourse import bacc
from concourse.bass_utils import run_bass_kernel_spmd
from concourse.masks import make_upper_triangular

F32 = mybir.dt.float32
F32R = mybir.dt.float32r
BF16 = mybir.dt.bfloat16

MODE = "bf16"            # "bf16" | "f32r"

B, S, D = 2, 2048, 1024
H, DH = 16, 64
NCORES = 8
GROUPS = 4               # head-groups (tensor parallel)
HG = H // GROUPS         # 4 heads per group
FEAT = HG * DH           # 256 features per group
SCALE = 1.0 / 8.0        # 1/sqrt(DH), folded into Wq/bq on host

CHUNK = 512              # seq chunk (PSUM bank = 512 fp32)
NSUB = CHUNK // 128      # 4 seq subtiles per chunk
NCHUNK = S // CHUNK      # 4
KD = D // 128            # 8 k-tiles over D
MT = FEAT // 128         # 2 feature M-tiles per group


def _emit(tc, mode):
    nc = tc.nc
    WDT = BF16 if mode == "bf16" else F32R
    if mode == "bf16":
        x = nc.dram_tensor("x", [S, D], BF16, kind="ExternalInput").ap()
    else:
        x_hi = nc.dram_tensor("x_hi", [S, D], BF16, kind="ExternalInput").ap()
        x_lo = nc.dram_tensor("x_lo", [S, D], BF16, kind="ExternalInput").ap()
    wq = nc.dram_tensor("wq", [D, FEAT], WDT, kind="ExternalInput").ap()
    wk = nc.dram_tensor("wk", [D, FEAT], WDT, kind="ExternalInput").ap()
    wv = nc.dram_tensor("wv", [D, FEAT], WDT, kind="ExternalInput").ap()
    bq = nc.dram_tensor("bq", [FEAT], F32, kind="ExternalInput").ap()
    bk = nc.dram_tensor("bk", [FEAT], F32, kind="ExternalInput").ap()
    bv = nc.dram_tensor("bv", [FEAT], F32, kind="ExternalInput").ap()
    wo = nc.dram_tensor("wo", [FEAT, D], WDT, kind="ExternalInput").ap()
    out = nc.dram_tensor("out", [S, D], F32, kind="ExternalOutput").ap()

    consts = tc.alloc_tile_pool(name="consts", bufs=1)
    weights = tc.alloc_tile_pool(name="weights", bufs=1)
    persist = tc.alloc_tile_pool(name="persist", bufs=1)
    xt_pool = tc.alloc_tile_pool(name="xt", bufs=3)
    et_pool = tc.alloc_tile_pool(name="et", bufs=6)
    rc_pool = tc.alloc_tile_pool(name="rc", bufs=3)
    ob_pool = tc.alloc_tile_pool(name="ob", bufs=2)
    work_ps = tc.alloc_tile_pool(name="work_ps", bufs=6, space="PSUM")
    cx_ps = tc.alloc_tile_pool(name="cx_ps", bufs=2, space="PSUM")

    # constants
    onesf = consts.tile([128, 64], F32)   # f32 scratch (memset can't write f32r)
    nc.vector.memset(onesf, 1.0)
    # tri[k, q] = 1 if q >= k else 0 (f32r memset is unsupported -> f32 there)
    tri = consts.tile([128, 128], BF16 if mode == "bf16" else F32)
    make_upper_triangular(nc, tri, val=1.0, diag=True)

    # weights
    wq_sb = weights.tile([128, KD, MT, 128], WDT)
    nc.sync.dma_start(wq_sb, wq.rearrange("(k p) (m f) -> p k m f", p=128, f=128))
    wk_sb = weights.tile([128, KD, MT, 128], WDT)
    nc.sync.dma_start(wk_sb, wk.rearrange("(k p) (m f) -> p k m f", p=128, f=128))
    wv_sb = weights.tile([128, KD, FEAT], WDT)
    nc.sync.dma_start(wv_sb, wv.rearrange("(k p) f -> p k f", p=128))
    wo_sb = weights.tile([128, MT, D], WDT)
    nc.sync.dma_start(wo_sb, wo.rearrange("(k p) n -> p k n", p=128))
    bqt = weights.tile([128, MT], F32)
    nc.sync.dma_start(bqt, bq.rearrange("(m p) -> p m", p=128))
    bkt = weights.tile([128, MT], F32)
    nc.sync.dma_start(bkt, bk.rearrange("(m p) -> p m", p=128))
    bvb = weights.tile([128, HG, DH], F32)
    nc.sync.dma_start(bvb, bv[None, :].to_broadcast([128, FEAT]).rearrange(
        "p (h f) -> p h f", h=HG))

    # persistent activations
    qt = persist.tile([128, MT, S], WDT)     # Q^T (features on partitions)
    # K^T stored per head in a full 128-partition tile: head h occupies rows
    # 64*(h%2)..+64 (matching qt's packing); the other 64 rows stay ZERO so
    # the scores matmul can contract over K=128 (keeps the full PE array
    # active for HAM) without changing the result.
    kt4 = persist.tile([128, HG, S], WDT)
    nc.vector.memset(kt4, 0.0)
    vaug = persist.tile([128, S // 128, HG, DH + 1], WDT)  # [V_h | 1] per head
    ctxT = persist.tile([128, MT, S], WDT)   # normalized ctx^T
    nc.vector.tensor_copy(vaug[:, :, :, DH],
                          onesf.rearrange("p (a b) -> p a b", a=S // 128))

    def normalize(c, h, cxt):
        """recip(denom), broadcast across partitions on GPSIMD
        (keeps the PE entirely out of the normalization chain), scale ctx."""
        cs = c * CHUNK
        ht, hr = h // 2, 64 * (h % 2)
        rc0 = rc_pool.tile([1, CHUNK], F32, tag="rc0")
        nc.vector.tensor_copy(rc0, cxt[DH:DH + 1, :])
        rc = rc_pool.tile([1, CHUNK], F32, tag="rc")
        nc.vector.reciprocal_approx_fast(rc, rc0)
        bcs = rc_pool.tile([64, CHUNK], F32, tag="bcs")
        nc.gpsimd.partition_broadcast(bcs, rc)
        nc.vector.tensor_mul(ctxT[hr:hr + 64, ht, cs:cs + CHUNK],
                             cxt[0:DH, :], bcs)

    def outproj(c):
        for t in range(NSUB):
            gt = c * NSUB + t
            ob = ob_pool.tile([128, D], F32)
            for n in range(D // 512):
                op = work_ps.tile([128, CHUNK], F32, tag="w")
                for k in range(MT):
                    nc.tensor.matmul(
                        op,
                        ctxT[:, k, gt * 128:(gt + 1) * 128],
                        wo_sb[:, k, 512 * n:512 * (n + 1)],
                        start=(k == 0), stop=(k == MT - 1))
                nc.vector.tensor_copy(ob[:, 512 * n:512 * (n + 1)], op)
            nc.sync.dma_start(out[gt * 128:(gt + 1) * 128, :], ob)

    for c in range(NCHUNK):
        cs = c * CHUNK
        # ---- load x^T chunk via DMA transpose ----
        if mode == "bf16":
            xt = xt_pool.tile([128, KD, CHUNK], BF16, tag="xt")
            for k in range(KD):
                nc.sync.dma_start_transpose(
                    xt[:, k, :], x[cs:cs + CHUNK, 128 * k:128 * (k + 1)])
        else:
            xh = xt_pool.tile([128, KD, CHUNK], BF16, tag="xh")
            xl = xt_pool.tile([128, KD, CHUNK], BF16, tag="xl")
            for k in range(KD):
                nc.sync.dma_start_transpose(
                    xh[:, k, :], x_hi[cs:cs + CHUNK, 128 * k:128 * (k + 1)])
                nc.sync.dma_start_transpose(
                    xl[:, k, :], x_lo[cs:cs + CHUNK, 128 * k:128 * (k + 1)])
            xt = xt_pool.tile([128, KD, CHUNK], F32R, tag="xt")
            for k in range(KD):
                nc.vector.tensor_add(xt[:, k, :], xh[:, k, :], xl[:, k, :])

        def proj_qkt(w_sb, bias_t, dst):
            for m in range(MT):
                ps = work_ps.tile([128, CHUNK], F32, tag="w", name="ps")
                for k in range(KD):
                    nc.tensor.matmul(ps, w_sb[:, k, m, :], xt[:, k, :],
                                     start=(k == 0), stop=(k == KD - 1))
                if dst is not None:
                    nc.scalar.activation(
                        dst[:, m, cs:cs + CHUNK], ps,
                        mybir.ActivationFunctionType.Identity,
                        bias=bias_t[:, m:m + 1], scale=1.0)
                else:
                    for hh in range(2):  # kt4: per-head halves, same rows
                        nc.scalar.activation(
                            kt4[64 * hh:64 * hh + 64, 2 * m + hh, cs:cs + CHUNK],
                            ps[64 * hh:64 * hh + 64, :],
                            mybir.ActivationFunctionType.Identity,
                            bias=bias_t[64 * hh:64 * hh + 64, m:m + 1], scale=1.0)

        def proj_v():
            # V projection (seq on partitions), augmented with ones col
            for t in range(NSUB):
                gt = c * NSUB + t
                ps = work_ps.tile([128, CHUNK], F32, tag="w", name="ps")
                for k in range(KD):
                    nc.tensor.matmul(ps[:, 0:FEAT],
                                     xt[:, k, t * 128:(t + 1) * 128],
                                     wv_sb[:, k, :],
                                     start=(k == 0), stop=(k == KD - 1))
                nc.vector.tensor_add(
                    vaug[:, gt, :, 0:DH],
                    ps[:, 0:FEAT].rearrange("p (h f) -> p h f", h=HG), bvb)

        jmax = c * NSUB + NSUB - 1

        def attn(h, cx, j0, j1, first, last):
            ht = h // 2
            for j in range(j0, j1):
                lv = max(0, 128 * j - cs)   # first valid q (chunk-local)
                nq = CHUNK - lv
                sp = work_ps.tile([128, CHUNK], F32, tag="w", name="sp")
                nc.tensor.matmul(sp[:, 0:nq],
                                 kt4[:, h, 128 * j:128 * (j + 1)],
                                 qt[:, ht, cs + lv:cs + CHUNK])
                et = et_pool.tile([128, CHUNK], WDT, name="et")
                nc.scalar.activation(et[:, 0:nq], sp[:, 0:nq],
                                     mybir.ActivationFunctionType.Exp)
                if j >= c * NSUB:  # diagonal block: causal triangular mask
                    nc.vector.tensor_mul(et[:, 0:128], et[:, 0:128], tri)
                nc.tensor.matmul(cx[:, lv:CHUNK], vaug[:, j, h, :],
                                 et[:, 0:nq],
                                 start=(first and j == j0),
                                 stop=(last and j == j1 - 1),
                                 skip_group_check=True)

        # Emission order: QT first, then head 0's off-diagonal attention (it
        # only needs qt + prior chunks' kt/v), with KT/V projections emitted
        # in between so the chunk-start exp never stalls the PE; normalizes
        # are PE-free, and each chunk's out-projection is deferred into the
        # next chunk.
        proj_qkt(wq_sb, bqt, qt)
        cx0 = cx_ps.tile([DH + 1, CHUNK], F32, tag="cx", name="cx0")
        attn(0, cx0, 0, c * NSUB, True, False)
        proj_qkt(wk_sb, bkt, None)
        proj_v()
        attn(0, cx0, c * NSUB, jmax + 1, c == 0, True)
        normalize(c, 0, cx0)
        for h in range(1, HG):
            cx = cx_ps.tile([DH + 1, CHUNK], F32, tag="cx", name="cx")
            attn(h, cx, 0, jmax + 1, True, True)
            normalize(c, h, cx)
            if h == 1 and c > 0:
                outproj(c - 1)

    outproj(NCHUNK - 1)

    for p in (cx_ps, work_ps, ob_pool, rc_pool, et_pool, xt_pool,
              persist, weights, consts):
        p.release()


_BUILT = None


def _build():
    global _BUILT
    if _BUILT is None:
        nc = bacc.Bacc("TRN2", target_bir_lowering=False, debug=False,
                       num_devices=NCORES)
        with tile.TileContext(nc) as tc:
            _emit(tc, MODE)
        nc.compile()
        _BUILT = nc
    return _BUILT


def _bf16(a):
    return np.ascontiguousarray(np.asarray(a, dtype=np.float32)).astype(
        ml_dtypes.bfloat16)


def _f32(a):
    return np.ascontiguousarray(np.asarray(a, dtype=np.float32))


def _shards(inputs):
    x = np.asarray(inputs["x"], dtype=np.float32)
    maps = []
    for core in range(NCORES):
        b, g = core // GROUPS, core % GROUPS
        f0 = g * FEAT
        m = {
            "bq": _f32(np.asarray(inputs["bq"], np.float32)[f0:f0 + FEAT] * SCALE),
            "bk": _f32(np.asarray(inputs["bk"], np.float32)[f0:f0 + FEAT]),
            "bv": _f32(np.asarray(inputs["bv"], np.float32)[f0:f0 + FEAT]),
        }
        wq_s = np.asarray(inputs["Wq"], np.float32)[:, f0:f0 + FEAT] * SCALE
        wk_s = np.asarray(inputs["Wk"], np.float32)[:, f0:f0 + FEAT]
        wv_s = np.asarray(inputs["Wv"], np.float32)[:, f0:f0 + FEAT]
        wo_s = np.asarray(inputs["Wo"], np.float32)[f0:f0 + FEAT, :]
        if MODE == "bf16":
            m["x"] = _bf16(x[b])
            m["wq"], m["wk"], m["wv"], m["wo"] = (
                _bf16(wq_s), _bf16(wk_s), _bf16(wv_s), _bf16(wo_s))
        else:
            hi = x[b].astype(ml_dtypes.bfloat16)
            lo = (x[b] - hi.astype(np.float32)).astype(ml_dtypes.bfloat16)
            m["x_hi"] = np.ascontiguousarray(hi)
            m["x_lo"] = np.ascontiguousarray(lo)
            m["wq"], m["wk"], m["wv"], m["wo"] = (
                _f32(wq_s), _f32(wk_s), _f32(wv_s), _f32(wo_s))
        maps.append(m)
    return maps


def kernel(trace=False, **inputs):
    nc = _build()
    res = run_bass_kernel_spmd(nc, _shards(inputs), core_ids=list(range(NCORES)),
                               trace=trace)
    partial = np.stack([r_["out"] for r_ in res.results])  # [8, S, D]
    acc = partial.reshape(B, GROUPS, S, D).astype(np.float64).sum(axis=1)
    acc += np.asarray(inputs["bo"], dtype=np.float64)
    out = acc.astype(np.float32)
    if trace:
        return out, res
    return out
